# revision 2
# baseline (speedup 1.0000x reference)
"""Trainium2 Bass kernel for nn_Attention_33157147525297 (v2 pipeline).

Graph-mixed multi-head attention, B=64, N=196 tokens, D=768, H=12 heads.
Data-parallel over batch: 8 batches per NeuronCore x 8 cores.

Math (host side): G mixes the query index only, so
  softmax(G (q k^T s)) v  ==  softmax((G_s q) k^T) v,  G_s = scale*G,
and G_s q = (G_s x) Wq^T, so the graph mix collapses to xg = G_s @ x.

Structure (cost model charges out_free_size * 0.417ns/col per matmul,
independent of K/M fill -- minimize total streamed columns, ~337k here):
  - x^T is pre-transposed on HOST (layout prep only); stage A computes just
    xg^T = x^T G_s^T via lhsT=x (token-major), rhs=G_s^T, two batches per
    psum tile so the psum->sbuf handoff stays off the PE critical path.
  - k^T = Wk x^T and q'^T = Wq xg^T feature-major (1568-token streams).
  - Attention: S^T = k q'^T per head ([128+68 j-tiles, 196]); exp on Act;
    PV token-major with P^T as lhsT streaming only 65 cols (64 v-cols + a
    ones column that yields the softmax sums), so the softmax scale is a
    per-partition multiply: one strided reciprocal + one stride-0-broadcast
    tensor_mul per 6-head psum bank.  No broadcast/ones matmuls at all.
  - O (token-major) is transposed back on the PE via identity matmuls
    (out_free=tsz) for the projection.
  - Projection D is tiled 13x128 over tokens PACKED ACROSS BATCHES (DRAM
    rows are contiguous), each tile DMA-ing out in 1-2 per-batch pieces.
Scheduling: dedicated psum pools decouple the S->exp chain from the GEMM
pipeline (fill/s/po/ot = 2/2/2/2 banks); v(2..7), deferred O-transposes and
D token-tiles interleave between attention pairs (2 filler units per pair)
to keep the PE fed while exp chains complete.  Engine split: exp + half the
copies on Act, GEMM copies/normalize/bias on DVE.

Infra notes: this container's walrus accepts only ONE attached semaphore
wait per instruction -- _install_wait_split() hoists extra waits onto
standalone EventSemaphore instructions.  Timing is the concourse TimelineSim
cost model (NTFF profiling unavailable under this axon client): 178549 ns
vs 205577 ns for the v1 kernel.
"""
import os
import sys
import numpy as np
import ml_dtypes

sys.path.insert(0, "/opt/trn_rl_repo")

SIZE, N_TOK, DIM, HEADS, HEAD_DIM, BATCH = 14, 196, 768, 12, 64, 64
N_CORES = 8
B_PER_CORE = BATCH // N_CORES  # 8
NT2 = 2 * N_TOK  # 392
NTB = N_TOK * B_PER_CORE  # 1568
BF16 = ml_dtypes.bfloat16

TOK_TILES = [(0, 128), (128, 68)]  # token-dim partition tiles (196 = 128+68)

LAST_EXEC_NS = None
LAST_TRACE = None


def _grid_g(factors):
    idx = np.arange(SIZE * SIZE).reshape(SIZE, SIZE)
    A = np.zeros((N_TOK, N_TOK), dtype=np.float32)
    for di, dj in [(-1, 0), (1, 0), (0, -1), (0, 1)]:
        for i in range(SIZE):
            for j in range(SIZE):
                ii, jj = i + di, j + dj
                if 0 <= ii < SIZE and 0 <= jj < SIZE:
                    A[idx[i, j], idx[ii, jj]] = 1.0
    NN = A / (A.sum(axis=1, keepdims=True) + 1.0)
    C = np.eye(N_TOK, dtype=np.float32) / 2.0
    return factors[0] * C + factors[1] * NN


def _install_wait_split():
    """This container's walrus rejects >1 attached semaphore wait per
    instruction ("Too many sync wait commands").  Hoist excess waits onto
    standalone InstEventSemaphore instructions just before, on the same
    engine — engine queues are in-order, so semantics are identical."""
    import concourse.mybir as mybir
    import concourse.tile as tile
    from concourse.vector_clock import ScopedClock

    TC = tile.TileContext
    if getattr(TC, "_wait_split_patched", False):
        return
    LIMIT = 1

    def _split(tc, inst):
        si = inst.sync_info
        if (si is None or not si.on_wait or len(si.on_wait) <= LIMIT
                or inst.engine == mybir.EngineType.Unassigned):
            return
        waits = list(si.on_wait)
        extra, keep = waits[:-LIMIT], waits[-LIMIT:]
        for i, w in enumerate(extra):
            ev = mybir.InstEventSemaphore(
                name=f"{inst.name}-ws{i}", engine=inst.engine,
                sync_info=mybir.SyncInfo(on_wait=[w], on_update=[]),
            )
            tc._add_instruction(ev)
        inst.sync_info = mybir.SyncInfo(on_wait=keep,
                                        on_update=list(si.on_update))

    orig_commit = TC._commit_instruction

    def patched_commit(self, inst, lazy_reg_writes=True):
        _split(self, inst)
        return orig_commit(self, inst, lazy_reg_writes=lazy_reg_writes)

    TC._commit_instruction = patched_commit

    def patched_drain_and_barrier(self, tick_clock, wait_clock):
        nc = self.nc
        probe = mybir.InstNoOp(
            name=f"drain-probe-{nc.next_id()}", engine=mybir.EngineType.SP)
        wait_clock.add_sem_waits(
            probe, ScopedClock({None: tick_clock.global_clock}))
        pw = probe.sync_info.on_wait if probe.sync_info else []
        for i, w in enumerate(pw):
            ev = mybir.InstEventSemaphore(
                name=f"drainw-{nc.next_id()}-{i}", engine=mybir.EngineType.SP,
                sync_info=mybir.SyncInfo(on_wait=[w], on_update=[]),
            )
            self._add_instruction(ev)
        nc.sync.drain()
        nc.all_engine_barrier()
        assert self.sems is not None
        popped = nc._tile_sem_poison_stack.pop()
        assert popped is self._sem_poison
        nc.clear_and_free_semaphores(list(self.sems.allocated().values()))
        nc.all_engine_barrier()

    TC._drain_and_barrier = patched_drain_and_barrier
    TC._wait_split_patched = True


def _build_bass():
    import concourse.bass as bass
    import concourse.mybir as mybir
    import concourse.tile as tile

    _install_wait_split()

    f32 = mybir.dt.float32
    bf16 = mybir.dt.bfloat16
    AF = mybir.ActivationFunctionType

    nc = bass.Bass()

    x_d = nc.declare_dram_parameter("x", [B_PER_CORE, N_TOK, DIM], bf16, isOutput=False)
    xT_d = nc.declare_dram_parameter("xT", [DIM, NTB], bf16, isOutput=False)
    gT_d = nc.declare_dram_parameter("gT", [N_TOK, N_TOK], bf16, isOutput=False)
    wq_d = nc.declare_dram_parameter("wqT", [DIM, DIM], bf16, isOutput=False)
    wk_d = nc.declare_dram_parameter("wkT", [DIM, DIM], bf16, isOutput=False)
    wv_d = nc.declare_dram_parameter("wvT", [DIM, DIM], bf16, isOutput=False)
    wp_d = nc.declare_dram_parameter("wpT", [DIM, DIM], bf16, isOutput=False)
    bias_d = nc.declare_dram_parameter("bias", [DIM], f32, isOutput=False)
    idn_d = nc.declare_dram_parameter("idn", [128, 128], bf16, isOutput=False)
    out_d = nc.declare_dram_parameter(
        "out", [B_PER_CORE, N_TOK, DIM], f32, isOutput=True
    )

    with tile.TileContext(nc) as tc:
        with (
            tc.tile_pool(name="const", bufs=1) as const_p,
            tc.tile_pool(name="big", bufs=1) as big_p,
            tc.tile_pool(name="tok", bufs=16) as tok_p,   # x then o_tok
            tc.tile_pool(name="cp", bufs=10) as cp_p,
            tc.tile_pool(name="rsp", bufs=8) as rs_p,
            tc.tile_pool(name="yp", bufs=4) as y_p,
            tc.tile_pool(name="ps_big", bufs=2, space="PSUM") as ps_big,
            tc.tile_pool(name="ps_s", bufs=2, space="PSUM") as ps_s,
            tc.tile_pool(name="ps_po", bufs=2, space="PSUM") as ps_po,
            tc.tile_pool(name="ps_ot", bufs=2, space="PSUM") as ps_ot,
        ):
            # ---- input DMAs (k-GEMM inputs first so PE starts ASAP;
            #      xT in nt-column chunks so k groups start after chunk 0) ----
            def load_w(d, nm, tiles=None):
                ts = []
                for kt in range(6):
                    t = const_p.tile([128, DIM], bf16, name=f"{nm}{kt}")
                    if tiles is None:
                        nc.sync.dma_start(out=t, in_=d[kt * 128:(kt + 1) * 128, :])
                    ts.append(t)
                return ts

            g_sb = []
            for ti, (t0, tsz) in enumerate(TOK_TILES):
                t = const_p.tile([128, N_TOK], bf16, name=f"g{ti}")
                nc.sync.dma_start(out=t[:tsz], in_=gT_d[t0:t0 + tsz, :])
                g_sb.append(t)

            x_sb = [[None, None] for _ in range(B_PER_CORE)]

            def load_x(b):
                for ti, (t0, tsz) in enumerate(TOK_TILES):
                    t = tok_p.tile([128, DIM], bf16, name=f"x{b}_{ti}", tag="tok")
                    nc.sync.dma_start(out=t[:tsz], in_=x_d[b, t0:t0 + tsz, :])
                    x_sb[b][ti] = t

            load_x(0)
            load_x(1)

            wk_sb = load_w(wk_d, "wk", tiles=False)
            xT_sb = [const_p.tile([128, NTB], bf16, name=f"xT{kt}")
                     for kt in range(6)]
            for kt in range(6):
                nc.sync.dma_start(out=wk_sb[kt],
                                  in_=wk_d[kt * 128:(kt + 1) * 128, :])
                nc.sync.dma_start(
                    out=xT_sb[kt][:, 0:NT2],
                    in_=xT_d[kt * 128:(kt + 1) * 128, 0:NT2])
            for nt in range(1, 4):
                for kt in range(6):
                    nc.sync.dma_start(
                        out=xT_sb[kt][:, nt * NT2:(nt + 1) * NT2],
                        in_=xT_d[kt * 128:(kt + 1) * 128,
                                 nt * NT2:(nt + 1) * NT2])
            for b in range(2, B_PER_CORE):
                load_x(b)

            wq_sb = load_w(wq_d, "wq")
            wv_sb = load_w(wv_d, "wv")
            wp_sb = load_w(wp_d, "wp")
            bias_sb = const_p.tile([128, DIM], f32, name="bias")
            nc.sync.dma_start(out=bias_sb,
                              in_=bias_d[None, :].broadcast_to([128, DIM]))
            idn_sb = const_p.tile([128, 128], bf16, name="idn")
            nc.sync.dma_start(out=idn_sb, in_=idn_d[:, :])

            # ---- persistent activations ----
            xg_sb = [big_p.tile([128, NTB], bf16, name=f"xg{k}")
                     for k in range(6)]
            qT_sb = [big_p.tile([128, NTB], bf16, name=f"qT{k}")
                     for k in range(6)]
            kT_sb = [big_p.tile([128, NTB], bf16, name=f"kT{k}")
                     for k in range(6)]
            oT_sb = [big_p.tile([128, NTB], bf16, name=f"oT{k}")
                     for k in range(6)]
            # v: token-major, 12 heads x 65 cols (col 64 of each = ones)
            v_sb = [
                [big_p.tile([128, 780], bf16, name=f"v{b}_{ti}") for ti in range(2)]
                for b in range(B_PER_CORE)
            ]
            o_tok = [[None, None] for _ in range(B_PER_CORE)]  # token-major O

            # ---- stage B-k: k^T = Wk @ x^T (feature-major) ----
            def k_group(mt, nt):
                ps = ps_big.tile([128, NT2], f32, tag="big", name="ps")
                for kt in range(6):
                    nc.tensor.matmul(
                        ps, wk_sb[kt][:, mt * 128:(mt + 1) * 128],
                        xT_sb[kt][:, nt * NT2:(nt + 1) * NT2],
                        start=(kt == 0), stop=(kt == 5),
                    )
                nc.vector.tensor_copy(kT_sb[mt][:, nt * NT2:(nt + 1) * NT2], ps)

            # ---- stage A: xg^T[d,i] = sum_j x[j,d] G_s^T[j,i] ----
            # two batches per psum tile: one copy per two iters so the
            # psum->sbuf handoff latency stays off the PE critical path
            def a_iter2(bp, kt):
                ps = ps_big.tile([128, NT2], f32, tag="big", name="ps")
                for sub in range(2):
                    b = 2 * bp + sub
                    for ti, (t0, tsz) in enumerate(TOK_TILES):
                        nc.tensor.matmul(
                            ps[:, sub * N_TOK:(sub + 1) * N_TOK],
                            x_sb[b][ti][:tsz, kt * 128:(kt + 1) * 128],
                            g_sb[ti][:tsz],
                            start=(ti == 0), stop=(ti == 1),
                        )
                dst = xg_sb[kt][:, 2 * bp * N_TOK:(2 * bp + 2) * N_TOK]
                if (bp + kt) % 2 == 0:
                    nc.scalar.activation(dst, ps, AF.Copy)
                else:
                    nc.vector.tensor_copy(dst, ps)

            # ---- stage B-q: q'^T = Wq @ xg^T ----
            def q_group(mt, nt):
                ps = ps_big.tile([128, NT2], f32, tag="big", name="ps")
                for kt in range(6):
                    nc.tensor.matmul(
                        ps, wq_sb[kt][:, mt * 128:(mt + 1) * 128],
                        xg_sb[kt][:, nt * NT2:(nt + 1) * NT2],
                        start=(kt == 0), stop=(kt == 5),
                    )
                nc.vector.tensor_copy(qT_sb[mt][:, nt * NT2:(nt + 1) * NT2], ps)

            # ---- stage B-v: v token-major with interleaved ones cols ----
            def v_unit(b, ti, nt):
                t0, tsz = TOK_TILES[ti]
                ps = ps_big.tile([128, NT2], f32, tag="big", name="ps")
                for kt in range(6):
                    nc.tensor.matmul(
                        ps[:tsz, :384],
                        xT_sb[kt][:, b * N_TOK + t0:b * N_TOK + t0 + tsz],
                        wv_sb[kt][:, nt * 384:(nt + 1) * 384],
                        start=(kt == 0), stop=(kt == 5),
                    )
                dst = v_sb[b][ti].rearrange("p (h c) -> p h c", h=12)
                nc.vector.tensor_copy(
                    dst[:tsz, nt * 6:(nt + 1) * 6, 0:64],
                    ps[:tsz, :384].rearrange("p (h c) -> p h c", h=6))
                if nt == 0:
                    nc.vector.memset(dst[:tsz, :, 64:65], 1.0)

            # ---- stage C: attention per (batch, head-pair) ----
            # po bank (b, mi, half): [tszi, 390] = heads 6*half..6*half+5,
            # 65 cols each (col 64 = softmax sums).
            po_banks = {}

            def c_pair(b, p):
                c0 = b * N_TOK
                half, hh = p // 3, None
                if p % 3 == 0:
                    for mi, (m0, msz) in enumerate(TOK_TILES):
                        po_banks[(b, mi, half)] = ps_po.tile(
                            [128, 390], f32, tag="po", name=f"po{b}_{mi}_{half}")
                pTs = []
                for hi in range(2):
                    h = 2 * p + hi
                    hb = hi * 64
                    s_ps = ps_s.tile([128, NT2], f32, tag="s", name="s")
                    for ti, (t0, tsz) in enumerate(TOK_TILES):
                        nc.tensor.matmul(
                            s_ps[:tsz, ti * N_TOK:(ti + 1) * N_TOK],
                            kT_sb[p][hb:hb + 64, c0 + t0:c0 + t0 + tsz],
                            qT_sb[p][hb:hb + 64, c0:c0 + N_TOK],
                            start=True, stop=True,
                        )
                    pT = cp_p.tile([128, NT2], bf16, tag="pT")
                    nc.scalar.activation(pT, s_ps, AF.Exp)
                    pTs.append(pT)
                for hi in range(2):
                    h = 2 * p + hi
                    hh = h - 6 * half
                    pT = pTs[hi]
                    for mi, (m0, msz) in enumerate(TOK_TILES):
                        po = po_banks[(b, mi, half)]
                        for ti, (t0, tsz) in enumerate(TOK_TILES):
                            nc.tensor.matmul(
                                po[:msz, 65 * hh:65 * hh + 65],
                                pT[:tsz, ti * N_TOK + m0:ti * N_TOK + m0 + msz],
                                v_sb[b][ti][:tsz, 65 * h:65 * h + 65],
                                start=(ti == 0), stop=(ti == 1),
                            )
                if p % 3 == 2:
                    # normalize heads 6*half..6*half+5 into o_tok
                    for mi, (m0, msz) in enumerate(TOK_TILES):
                        if half == 0 and o_tok[b][mi] is None:
                            o_tok[b][mi] = tok_p.tile(
                                [128, DIM], bf16, name=f"o{b}_{mi}", tag="tok")
                        po = po_banks.pop((b, mi, half))
                        pv = po.rearrange("p (h c) -> p h c", h=6)
                        rs = rs_p.tile([128, 6], bf16, tag="rs")
                        with nc.allow_low_precision(reason="softmax recip"):
                            nc.vector.reciprocal(rs[:msz], pv[:msz, :, 64])
                            ov = o_tok[b][mi].rearrange(
                                "p (h c) -> p h c", h=12)
                            nc.vector.tensor_mul(
                                ov[:msz, 6 * half:6 * half + 6, :],
                                pv[:msz, :, 0:64],
                                rs[:msz, :, None].broadcast_to([msz, 6, 64]),
                            )
            # transpose a group of 2 o_tok column-tiles -> oT (feature-major)
            def t_group(b, g):
                c0 = b * N_TOK
                mi = g // 3
                m0, msz = TOK_TILES[mi]
                for j in range(2):
                    kt = (g % 3) * 2 + j
                    ot = ps_ot.tile([128, 128], bf16, tag="ot")
                    nc.tensor.transpose(
                        ot[:, :msz],
                        o_tok[b][mi][:msz, kt * 128:(kt + 1) * 128],
                        idn_sb[:msz, :msz],
                    )
                    if (kt + mi) % 2 == 0:
                        nc.vector.tensor_copy(
                            oT_sb[kt][:, c0 + m0:c0 + m0 + msz], ot[:, :msz])
                    else:
                        nc.scalar.activation(
                            oT_sb[kt][:, c0 + m0:c0 + m0 + msz], ot[:, :msz],
                            AF.Copy)

            # ---- stage D: y = O @ Wp^T + bias; DMA out ----
            # token tiles packed across batch boundaries (13 x 128 instead of
            # 8 x (128+68)): DRAM rows are contiguous over (b, t), so each
            # tile DMAs out in 1-2 per-batch pieces
            def d_unit(tt, ti, nt):
                t0 = tt * 128
                tsz = min(128, NTB - t0)
                ps = ps_big.tile([128, NT2], f32, tag="big", name="ps")
                for kt in range(6):
                    nc.tensor.matmul(
                        ps[:tsz, :384],
                        oT_sb[kt][:, t0:t0 + tsz],
                        wp_sb[kt][:, nt * 384:(nt + 1) * 384],
                        start=(kt == 0), stop=(kt == 5),
                    )
                y_sb = y_p.tile([128, 384], f32, tag="y", name="y_sb")
                nc.vector.tensor_add(
                    y_sb[:tsz], ps[:tsz, :384],
                    bias_sb[:tsz, nt * 384:(nt + 1) * 384])
                r0 = t0
                while r0 < t0 + tsz:
                    b = r0 // N_TOK
                    r1 = min((b + 1) * N_TOK, t0 + tsz)
                    nc.sync.dma_start(
                        out=out_d[b, r0 - b * N_TOK:r1 - b * N_TOK,
                                  nt * 384:(nt + 1) * 384],
                        in_=y_sb[r0 - t0:r1 - t0])
                    r0 = r1

            # ---- schedule ----
            for kt in range(6):
                a_iter2(0, kt)
            for nt in range(4):
                for mt in range(6):
                    k_group(mt, nt)
            for bp in range(1, B_PER_CORE // 2):
                for kt in range(6):
                    a_iter2(bp, kt)
            for nt in range(4):
                for mt in range(6):
                    q_group(mt, nt)
            for b in (0, 1):
                for ti in range(2):
                    for nt in range(2):
                        v_unit(b, ti, nt)

            # C with v(2..7), deferred transposes, and D(b) units
            # interleaved between pairs (2 fillers per pair).
            from collections import deque
            fillers = deque()
            for b in range(2, B_PER_CORE):
                for ti in range(2):
                    for nt in range(2):
                        fillers.append(("v", b, ti, nt))

            def pop_fill(n):
                for _ in range(n):
                    if not fillers:
                        return
                    kind, fb, i1, i2 = fillers.popleft()
                    if kind == "v":
                        v_unit(fb, i1, i2)
                    elif kind == "t":
                        t_group(fb, i1)
                    else:
                        d_unit(fb, i1, i2)

            for b in range(B_PER_CORE):
                for p in range(6):
                    c_pair(b, p)
                    pop_fill(2)
                for g in range(6):
                    fillers.append(("t", b, g, 0))
                # D token-tiles whose last contributing batch is b
                for tt in range(13):
                    tsz = min(128, NTB - tt * 128)
                    if (tt * 128 + tsz - 1) // N_TOK == b:
                        for nt in range(2):
                            fillers.append(("d", tt, 0, nt))
            pop_fill(10**9)

    return nc


_CACHED_NC = None


def kernel(x, w_qkv, w_proj, b_proj, factors):
    global LAST_EXEC_NS, LAST_TRACE, _CACHED_NC
    from concourse.bass_utils import run_bass_kernel_spmd

    factors = np.asarray(factors, dtype=np.float32)
    scale = HEAD_DIM ** -0.5
    G_s = _grid_g(factors) * scale

    w_qkv = np.asarray(w_qkv, dtype=np.float32)
    in_common = {
        "gT": np.ascontiguousarray(G_s.T).astype(BF16),
        "wqT": np.ascontiguousarray(w_qkv[0:DIM, :].T).astype(BF16),
        "wkT": np.ascontiguousarray(w_qkv[DIM:2 * DIM, :].T).astype(BF16),
        "wvT": np.ascontiguousarray(w_qkv[2 * DIM:3 * DIM, :].T).astype(BF16),
        "wpT": np.ascontiguousarray(np.asarray(w_proj, dtype=np.float32).T).astype(BF16),
        "bias": np.asarray(b_proj, dtype=np.float32),
        "idn": np.eye(128, dtype=np.float32).astype(BF16),
    }
    x = np.asarray(x, dtype=np.float32).astype(BF16)
    in_maps = []
    for c in range(N_CORES):
        xc = x[c * B_PER_CORE:(c + 1) * B_PER_CORE]
        xTc = np.ascontiguousarray(
            xc.reshape(NTB, DIM).T)
        in_maps.append({"x": xc, "xT": xTc, **in_common})

    if _CACHED_NC is None:
        _CACHED_NC = _build_bass()
    nc = _CACHED_NC

    trace = bool(int(os.environ.get("KERNEL_TRACE", "0")))
    res = run_bass_kernel_spmd(nc, in_maps, core_ids=list(range(N_CORES)),
                               trace=trace)
    LAST_EXEC_NS = res.exec_time_ns
    if res.instructions_and_trace is not None:
        LAST_TRACE = res.instructions_and_trace[1]
    out = np.concatenate([res.results[c]["out"] for c in range(N_CORES)], axis=0)
    return out.astype(np.float32)


# revision 3
# speedup vs baseline: 1.0394x; 1.0394x over previous
"""Trainium2 Bass kernel for nn_Attention_33157147525297 (v2 pipeline).

Graph-mixed multi-head attention, B=64, N=196 tokens, D=768, H=12 heads.
Data-parallel over batch: 8 batches per NeuronCore x 8 cores.

Math (host side): G mixes the query index only, so
  softmax(G (q k^T s)) v  ==  softmax((G_s q) k^T) v,  G_s = scale*G,
and G_s q = (G_s x) Wq^T, so the graph mix collapses to xg = G_s @ x.

Structure (cost model charges out_free_size * 0.417ns/col per matmul,
independent of K/M fill -- minimize total streamed columns, ~337k here):
  - x^T is pre-transposed on HOST (layout prep only); stage A computes just
    xg^T = x^T G_s^T via lhsT=x (token-major), rhs=G_s^T, two batches per
    psum tile so the psum->sbuf handoff stays off the PE critical path.
  - k^T = Wk x^T and q'^T = Wq xg^T feature-major (1568-token streams).
  - Attention: S^T = k q'^T per head ([128+68 j-tiles, 196]); exp on Act;
    PV token-major with P^T as lhsT streaming only 65 cols (64 v-cols + a
    ones column that yields the softmax sums), so the softmax scale is a
    per-partition multiply: one strided reciprocal + one stride-0-broadcast
    tensor_mul per 6-head psum bank.  No broadcast/ones matmuls at all.
  - O (token-major) is transposed back on the PE via identity matmuls
    (out_free=tsz) for the projection.
  - Projection D is tiled 13x128 over tokens PACKED ACROSS BATCHES (DRAM
    rows are contiguous), each tile DMA-ing out in 1-2 per-batch pieces.
Scheduling: dedicated psum pools decouple the S->exp chain from the GEMM
pipeline (fill/s/po/ot = 2/2/2/2 banks); v(2..7), deferred O-transposes and
D token-tiles interleave between attention pairs (1-2 filler units per
pair, throttled to keep inventory for the final batches) so the PE stays
fed while exp chains complete.  Engine split: exp + half the copies on Act,
GEMM copies/normalize/bias on DVE.

Infra notes: this container's walrus accepts only ONE attached semaphore
wait per instruction -- _install_wait_split() hoists extra waits onto
standalone EventSemaphore instructions.  Timing is the concourse TimelineSim
cost model (NTFF profiling unavailable under this axon client): 177792 ns
vs 205577 ns for the v1 kernel (-13.5%), rel err 3.4e-03.
"""
import os
import sys
import numpy as np
import ml_dtypes

sys.path.insert(0, "/opt/trn_rl_repo")

SIZE, N_TOK, DIM, HEADS, HEAD_DIM, BATCH = 14, 196, 768, 12, 64, 64
N_CORES = 8
B_PER_CORE = BATCH // N_CORES  # 8
NT2 = 2 * N_TOK  # 392
NTB = N_TOK * B_PER_CORE  # 1568
BF16 = ml_dtypes.bfloat16

TOK_TILES = [(0, 128), (128, 68)]  # token-dim partition tiles (196 = 128+68)

LAST_EXEC_NS = None
LAST_TRACE = None


def _grid_g(factors):
    idx = np.arange(SIZE * SIZE).reshape(SIZE, SIZE)
    A = np.zeros((N_TOK, N_TOK), dtype=np.float32)
    for di, dj in [(-1, 0), (1, 0), (0, -1), (0, 1)]:
        for i in range(SIZE):
            for j in range(SIZE):
                ii, jj = i + di, j + dj
                if 0 <= ii < SIZE and 0 <= jj < SIZE:
                    A[idx[i, j], idx[ii, jj]] = 1.0
    NN = A / (A.sum(axis=1, keepdims=True) + 1.0)
    C = np.eye(N_TOK, dtype=np.float32) / 2.0
    return factors[0] * C + factors[1] * NN


def _install_wait_split():
    """This container's walrus rejects >1 attached semaphore wait per
    instruction ("Too many sync wait commands").  Hoist excess waits onto
    standalone InstEventSemaphore instructions just before, on the same
    engine — engine queues are in-order, so semantics are identical."""
    import concourse.mybir as mybir
    import concourse.tile as tile
    from concourse.vector_clock import ScopedClock

    TC = tile.TileContext
    if getattr(TC, "_wait_split_patched", False):
        return
    LIMIT = 1

    def _split(tc, inst):
        si = inst.sync_info
        if (si is None or not si.on_wait or len(si.on_wait) <= LIMIT
                or inst.engine == mybir.EngineType.Unassigned):
            return
        waits = list(si.on_wait)
        extra, keep = waits[:-LIMIT], waits[-LIMIT:]
        for i, w in enumerate(extra):
            ev = mybir.InstEventSemaphore(
                name=f"{inst.name}-ws{i}", engine=inst.engine,
                sync_info=mybir.SyncInfo(on_wait=[w], on_update=[]),
            )
            tc._add_instruction(ev)
        inst.sync_info = mybir.SyncInfo(on_wait=keep,
                                        on_update=list(si.on_update))

    orig_commit = TC._commit_instruction

    def patched_commit(self, inst, lazy_reg_writes=True):
        _split(self, inst)
        return orig_commit(self, inst, lazy_reg_writes=lazy_reg_writes)

    TC._commit_instruction = patched_commit

    def patched_drain_and_barrier(self, tick_clock, wait_clock):
        nc = self.nc
        probe = mybir.InstNoOp(
            name=f"drain-probe-{nc.next_id()}", engine=mybir.EngineType.SP)
        wait_clock.add_sem_waits(
            probe, ScopedClock({None: tick_clock.global_clock}))
        pw = probe.sync_info.on_wait if probe.sync_info else []
        for i, w in enumerate(pw):
            ev = mybir.InstEventSemaphore(
                name=f"drainw-{nc.next_id()}-{i}", engine=mybir.EngineType.SP,
                sync_info=mybir.SyncInfo(on_wait=[w], on_update=[]),
            )
            self._add_instruction(ev)
        nc.sync.drain()
        nc.all_engine_barrier()
        assert self.sems is not None
        popped = nc._tile_sem_poison_stack.pop()
        assert popped is self._sem_poison
        nc.clear_and_free_semaphores(list(self.sems.allocated().values()))
        nc.all_engine_barrier()

    TC._drain_and_barrier = patched_drain_and_barrier
    TC._wait_split_patched = True


def _build_bass():
    import concourse.bass as bass
    import concourse.mybir as mybir
    import concourse.tile as tile

    _install_wait_split()

    f32 = mybir.dt.float32
    bf16 = mybir.dt.bfloat16
    AF = mybir.ActivationFunctionType

    nc = bass.Bass()

    x_d = nc.declare_dram_parameter("x", [B_PER_CORE, N_TOK, DIM], bf16, isOutput=False)
    xT_d = nc.declare_dram_parameter("xT", [DIM, NTB], bf16, isOutput=False)
    gT_d = nc.declare_dram_parameter("gT", [N_TOK, N_TOK], bf16, isOutput=False)
    wq_d = nc.declare_dram_parameter("wqT", [DIM, DIM], bf16, isOutput=False)
    wk_d = nc.declare_dram_parameter("wkT", [DIM, DIM], bf16, isOutput=False)
    wv_d = nc.declare_dram_parameter("wvT", [DIM, DIM], bf16, isOutput=False)
    wp_d = nc.declare_dram_parameter("wpT", [DIM, DIM], bf16, isOutput=False)
    bias_d = nc.declare_dram_parameter("bias", [DIM], f32, isOutput=False)
    idn_d = nc.declare_dram_parameter("idn", [128, 128], bf16, isOutput=False)
    out_d = nc.declare_dram_parameter(
        "out", [B_PER_CORE, N_TOK, DIM], f32, isOutput=True
    )

    with tile.TileContext(nc) as tc:
        with (
            tc.tile_pool(name="const", bufs=1) as const_p,
            tc.tile_pool(name="big", bufs=1) as big_p,
            tc.tile_pool(name="tok", bufs=16) as tok_p,   # x then o_tok
            tc.tile_pool(name="cp", bufs=10) as cp_p,
            tc.tile_pool(name="rsp", bufs=8) as rs_p,
            tc.tile_pool(name="yp", bufs=4) as y_p,
            tc.tile_pool(name="ps_big", bufs=2, space="PSUM") as ps_big,
            tc.tile_pool(name="ps_s", bufs=2, space="PSUM") as ps_s,
            tc.tile_pool(name="ps_po", bufs=2, space="PSUM") as ps_po,
            tc.tile_pool(name="ps_ot", bufs=2, space="PSUM") as ps_ot,
        ):
            # ---- input DMAs (k-GEMM inputs first so PE starts ASAP;
            #      xT in nt-column chunks so k groups start after chunk 0) ----
            def load_w(d, nm, tiles=None):
                ts = []
                for kt in range(6):
                    t = const_p.tile([128, DIM], bf16, name=f"{nm}{kt}")
                    if tiles is None:
                        nc.sync.dma_start(out=t, in_=d[kt * 128:(kt + 1) * 128, :])
                    ts.append(t)
                return ts

            g_sb = []
            for ti, (t0, tsz) in enumerate(TOK_TILES):
                t = const_p.tile([128, N_TOK], bf16, name=f"g{ti}")
                nc.sync.dma_start(out=t[:tsz], in_=gT_d[t0:t0 + tsz, :])
                g_sb.append(t)

            x_sb = [[None, None] for _ in range(B_PER_CORE)]

            def load_x(b):
                for ti, (t0, tsz) in enumerate(TOK_TILES):
                    t = tok_p.tile([128, DIM], bf16, name=f"x{b}_{ti}", tag="tok")
                    nc.sync.dma_start(out=t[:tsz], in_=x_d[b, t0:t0 + tsz, :])
                    x_sb[b][ti] = t

            load_x(0)
            load_x(1)

            wk_sb = load_w(wk_d, "wk", tiles=False)
            xT_sb = [const_p.tile([128, NTB], bf16, name=f"xT{kt}")
                     for kt in range(6)]
            for kt in range(6):
                nc.sync.dma_start(out=wk_sb[kt],
                                  in_=wk_d[kt * 128:(kt + 1) * 128, :])
                nc.sync.dma_start(
                    out=xT_sb[kt][:, 0:NT2],
                    in_=xT_d[kt * 128:(kt + 1) * 128, 0:NT2])
            for nt in range(1, 4):
                for kt in range(6):
                    nc.sync.dma_start(
                        out=xT_sb[kt][:, nt * NT2:(nt + 1) * NT2],
                        in_=xT_d[kt * 128:(kt + 1) * 128,
                                 nt * NT2:(nt + 1) * NT2])
            for b in range(2, B_PER_CORE):
                load_x(b)

            wq_sb = load_w(wq_d, "wq")
            wv_sb = load_w(wv_d, "wv")
            wp_sb = load_w(wp_d, "wp")
            bias_sb = const_p.tile([128, DIM], f32, name="bias")
            nc.sync.dma_start(out=bias_sb,
                              in_=bias_d[None, :].broadcast_to([128, DIM]))
            idn_sb = const_p.tile([128, 128], bf16, name="idn")
            nc.sync.dma_start(out=idn_sb, in_=idn_d[:, :])

            # ---- persistent activations ----
            xg_sb = [big_p.tile([128, NTB], bf16, name=f"xg{k}")
                     for k in range(6)]
            qT_sb = [big_p.tile([128, NTB], bf16, name=f"qT{k}")
                     for k in range(6)]
            kT_sb = [big_p.tile([128, NTB], bf16, name=f"kT{k}")
                     for k in range(6)]
            oT_sb = [big_p.tile([128, NTB], bf16, name=f"oT{k}")
                     for k in range(6)]
            # v: token-major, 12 heads x 65 cols (col 64 of each = ones)
            v_sb = [
                [big_p.tile([128, 780], bf16, name=f"v{b}_{ti}") for ti in range(2)]
                for b in range(B_PER_CORE)
            ]
            o_tok = [[None, None] for _ in range(B_PER_CORE)]  # token-major O

            # ---- stage B-k: k^T = Wk @ x^T (feature-major) ----
            def k_group(mt, nt):
                ps = ps_big.tile([128, NT2], f32, tag="big", name="ps")
                for kt in range(6):
                    nc.tensor.matmul(
                        ps, wk_sb[kt][:, mt * 128:(mt + 1) * 128],
                        xT_sb[kt][:, nt * NT2:(nt + 1) * NT2],
                        start=(kt == 0), stop=(kt == 5),
                    )
                nc.vector.tensor_copy(kT_sb[mt][:, nt * NT2:(nt + 1) * NT2], ps)

            # ---- stage A: xg^T[d,i] = sum_j x[j,d] G_s^T[j,i] ----
            # two batches per psum tile: one copy per two iters so the
            # psum->sbuf handoff latency stays off the PE critical path
            def a_iter2(bp, kt):
                ps = ps_big.tile([128, NT2], f32, tag="big", name="ps")
                for sub in range(2):
                    b = 2 * bp + sub
                    for ti, (t0, tsz) in enumerate(TOK_TILES):
                        nc.tensor.matmul(
                            ps[:, sub * N_TOK:(sub + 1) * N_TOK],
                            x_sb[b][ti][:tsz, kt * 128:(kt + 1) * 128],
                            g_sb[ti][:tsz],
                            start=(ti == 0), stop=(ti == 1),
                        )
                dst = xg_sb[kt][:, 2 * bp * N_TOK:(2 * bp + 2) * N_TOK]
                if (bp + kt) % 2 == 0:
                    nc.scalar.activation(dst, ps, AF.Copy)
                else:
                    nc.vector.tensor_copy(dst, ps)

            # ---- stage B-q: q'^T = Wq @ xg^T ----
            def q_group(mt, nt):
                ps = ps_big.tile([128, NT2], f32, tag="big", name="ps")
                for kt in range(6):
                    nc.tensor.matmul(
                        ps, wq_sb[kt][:, mt * 128:(mt + 1) * 128],
                        xg_sb[kt][:, nt * NT2:(nt + 1) * NT2],
                        start=(kt == 0), stop=(kt == 5),
                    )
                nc.vector.tensor_copy(qT_sb[mt][:, nt * NT2:(nt + 1) * NT2], ps)

            # ---- stage B-v: v token-major with interleaved ones cols ----
            def v_unit(b, ti, nt):
                t0, tsz = TOK_TILES[ti]
                ps = ps_big.tile([128, NT2], f32, tag="big", name="ps")
                for kt in range(6):
                    nc.tensor.matmul(
                        ps[:tsz, :384],
                        xT_sb[kt][:, b * N_TOK + t0:b * N_TOK + t0 + tsz],
                        wv_sb[kt][:, nt * 384:(nt + 1) * 384],
                        start=(kt == 0), stop=(kt == 5),
                    )
                dst = v_sb[b][ti].rearrange("p (h c) -> p h c", h=12)
                nc.vector.tensor_copy(
                    dst[:tsz, nt * 6:(nt + 1) * 6, 0:64],
                    ps[:tsz, :384].rearrange("p (h c) -> p h c", h=6))
                if nt == 0:
                    nc.vector.memset(dst[:tsz, :, 64:65], 1.0)

            # ---- stage C: attention per (batch, head-pair) ----
            # po bank (b, mi, half): [tszi, 390] = heads 6*half..6*half+5,
            # 65 cols each (col 64 = softmax sums).
            po_banks = {}

            def c_pair(b, p):
                c0 = b * N_TOK
                half, hh = p // 3, None
                if p % 3 == 0:
                    for mi, (m0, msz) in enumerate(TOK_TILES):
                        po_banks[(b, mi, half)] = ps_po.tile(
                            [128, 390], f32, tag="po", name=f"po{b}_{mi}_{half}")
                pTs = []
                for hi in range(2):
                    h = 2 * p + hi
                    hb = hi * 64
                    s_ps = ps_s.tile([128, NT2], f32, tag="s", name="s")
                    for ti, (t0, tsz) in enumerate(TOK_TILES):
                        nc.tensor.matmul(
                            s_ps[:tsz, ti * N_TOK:(ti + 1) * N_TOK],
                            kT_sb[p][hb:hb + 64, c0 + t0:c0 + t0 + tsz],
                            qT_sb[p][hb:hb + 64, c0:c0 + N_TOK],
                            start=True, stop=True,
                        )
                    pT = cp_p.tile([128, NT2], bf16, tag="pT")
                    nc.scalar.activation(pT, s_ps, AF.Exp)
                    pTs.append(pT)
                for hi in range(2):
                    h = 2 * p + hi
                    hh = h - 6 * half
                    pT = pTs[hi]
                    for mi, (m0, msz) in enumerate(TOK_TILES):
                        po = po_banks[(b, mi, half)]
                        for ti, (t0, tsz) in enumerate(TOK_TILES):
                            nc.tensor.matmul(
                                po[:msz, 65 * hh:65 * hh + 65],
                                pT[:tsz, ti * N_TOK + m0:ti * N_TOK + m0 + msz],
                                v_sb[b][ti][:tsz, 65 * h:65 * h + 65],
                                start=(ti == 0), stop=(ti == 1),
                            )
                if p % 3 == 2:
                    # normalize heads 6*half..6*half+5 into o_tok
                    for mi, (m0, msz) in enumerate(TOK_TILES):
                        if half == 0 and o_tok[b][mi] is None:
                            o_tok[b][mi] = tok_p.tile(
                                [128, DIM], bf16, name=f"o{b}_{mi}", tag="tok")
                        po = po_banks.pop((b, mi, half))
                        pv = po.rearrange("p (h c) -> p h c", h=6)
                        rs = rs_p.tile([128, 6], bf16, tag="rs")
                        with nc.allow_low_precision(reason="softmax recip"):
                            nc.vector.reciprocal(rs[:msz], pv[:msz, :, 64])
                            ov = o_tok[b][mi].rearrange(
                                "p (h c) -> p h c", h=12)
                            nc.vector.tensor_mul(
                                ov[:msz, 6 * half:6 * half + 6, :],
                                pv[:msz, :, 0:64],
                                rs[:msz, :, None].broadcast_to([msz, 6, 64]),
                            )
            # transpose a group of 2 o_tok column-tiles -> oT (feature-major)
            def t_group(b, g):
                c0 = b * N_TOK
                mi = g // 3
                m0, msz = TOK_TILES[mi]
                for j in range(2):
                    kt = (g % 3) * 2 + j
                    ot = ps_ot.tile([128, 128], bf16, tag="ot")
                    nc.tensor.transpose(
                        ot[:, :msz],
                        o_tok[b][mi][:msz, kt * 128:(kt + 1) * 128],
                        idn_sb[:msz, :msz],
                    )
                    if (kt + mi) % 2 == 0:
                        nc.vector.tensor_copy(
                            oT_sb[kt][:, c0 + m0:c0 + m0 + msz], ot[:, :msz])
                    else:
                        nc.scalar.activation(
                            oT_sb[kt][:, c0 + m0:c0 + m0 + msz], ot[:, :msz],
                            AF.Copy)

            # ---- stage D: y = O @ Wp^T + bias; DMA out ----
            # token tiles packed across batch boundaries (13 x 128 instead of
            # 8 x (128+68)): DRAM rows are contiguous over (b, t), so each
            # tile DMAs out in 1-2 per-batch pieces
            def d_unit(tt, ti, nt):
                t0 = tt * 128
                tsz = min(128, NTB - t0)
                ps = ps_big.tile([128, NT2], f32, tag="big", name="ps")
                for kt in range(6):
                    nc.tensor.matmul(
                        ps[:tsz, :384],
                        oT_sb[kt][:, t0:t0 + tsz],
                        wp_sb[kt][:, nt * 384:(nt + 1) * 384],
                        start=(kt == 0), stop=(kt == 5),
                    )
                y_sb = y_p.tile([128, 384], f32, tag="y", name="y_sb")
                nc.vector.tensor_add(
                    y_sb[:tsz], ps[:tsz, :384],
                    bias_sb[:tsz, nt * 384:(nt + 1) * 384])
                r0 = t0
                while r0 < t0 + tsz:
                    b = r0 // N_TOK
                    r1 = min((b + 1) * N_TOK, t0 + tsz)
                    nc.sync.dma_start(
                        out=out_d[b, r0 - b * N_TOK:r1 - b * N_TOK,
                                  nt * 384:(nt + 1) * 384],
                        in_=y_sb[r0 - t0:r1 - t0])
                    r0 = r1

            # ---- schedule ----
            for kt in range(6):
                a_iter2(0, kt)
            for nt in range(4):
                for mt in range(6):
                    k_group(mt, nt)
            for bp in range(1, B_PER_CORE // 2):
                for kt in range(6):
                    a_iter2(bp, kt)
            for nt in range(4):
                for mt in range(6):
                    q_group(mt, nt)
            for b in (0, 1):
                for ti in range(2):
                    for nt in range(2):
                        v_unit(b, ti, nt)

            # C with v(2..7), deferred transposes, and D(b) units
            # interleaved between pairs (2 fillers per pair).
            from collections import deque
            fillers = deque()
            for b in range(2, B_PER_CORE):
                for ti in range(2):
                    for nt in range(2):
                        fillers.append(("v", b, ti, nt))

            def pop_fill(n):
                for _ in range(n):
                    if not fillers:
                        return
                    kind, fb, i1, i2 = fillers.popleft()
                    if kind == "v":
                        v_unit(fb, i1, i2)
                    elif kind == "t":
                        t_group(fb, i1)
                    else:
                        d_unit(fb, i1, i2)

            for b in range(B_PER_CORE):
                for p in range(6):
                    c_pair(b, p)
                    pop_fill(2 if len(fillers) > 20 else 1)
                for g in range(6):
                    fillers.append(("t", b, g, 0))
                # D token-tiles whose last contributing batch is b
                for tt in range(13):
                    tsz = min(128, NTB - tt * 128)
                    if (tt * 128 + tsz - 1) // N_TOK == b:
                        for nt in range(2):
                            fillers.append(("d", tt, 0, nt))
            pop_fill(10**9)

    return nc


_CACHED_NC = None


def kernel(x, w_qkv, w_proj, b_proj, factors):
    global LAST_EXEC_NS, LAST_TRACE, _CACHED_NC
    from concourse.bass_utils import run_bass_kernel_spmd

    factors = np.asarray(factors, dtype=np.float32)
    scale = HEAD_DIM ** -0.5
    G_s = _grid_g(factors) * scale

    w_qkv = np.asarray(w_qkv, dtype=np.float32)
    in_common = {
        "gT": np.ascontiguousarray(G_s.T).astype(BF16),
        "wqT": np.ascontiguousarray(w_qkv[0:DIM, :].T).astype(BF16),
        "wkT": np.ascontiguousarray(w_qkv[DIM:2 * DIM, :].T).astype(BF16),
        "wvT": np.ascontiguousarray(w_qkv[2 * DIM:3 * DIM, :].T).astype(BF16),
        "wpT": np.ascontiguousarray(np.asarray(w_proj, dtype=np.float32).T).astype(BF16),
        "bias": np.asarray(b_proj, dtype=np.float32),
        "idn": np.eye(128, dtype=np.float32).astype(BF16),
    }
    x = np.asarray(x, dtype=np.float32).astype(BF16)
    in_maps = []
    for c in range(N_CORES):
        xc = x[c * B_PER_CORE:(c + 1) * B_PER_CORE]
        xTc = np.ascontiguousarray(
            xc.reshape(NTB, DIM).T)
        in_maps.append({"x": xc, "xT": xTc, **in_common})

    if _CACHED_NC is None:
        _CACHED_NC = _build_bass()
    nc = _CACHED_NC

    trace = bool(int(os.environ.get("KERNEL_TRACE", "0")))
    res = run_bass_kernel_spmd(nc, in_maps, core_ids=list(range(N_CORES)),
                               trace=trace)
    LAST_EXEC_NS = res.exec_time_ns
    if res.instructions_and_trace is not None:
        LAST_TRACE = res.instructions_and_trace[1]
    out = np.concatenate([res.results[c]["out"] for c in range(N_CORES)], axis=0)
    return out.astype(np.float32)


# revision 4
# speedup vs baseline: 1.0415x; 1.0020x over previous
"""Trainium2 Bass kernel for nn_Attention_33157147525297 (v2 pipeline).

Graph-mixed multi-head attention, B=64, N=196 tokens, D=768, H=12 heads.
Data-parallel over batch: 8 batches per NeuronCore x 8 cores.

Math (host side): G mixes the query index only, so
  softmax(G (q k^T s)) v  ==  softmax((G_s q) k^T) v,  G_s = scale*G,
and G_s q = (G_s x) Wq^T, so the graph mix collapses to xg = G_s @ x.

Structure (cost model charges out_free_size * 0.417ns/col per matmul,
independent of K/M fill -- minimize total streamed columns, ~337k here):
  - x^T is pre-transposed on HOST (layout prep only); stage A computes just
    xg^T = x^T G_s^T via lhsT=x (token-major), rhs=G_s^T, two batches per
    psum tile so the psum->sbuf handoff stays off the PE critical path.
  - k^T = Wk x^T and q'^T = Wq xg^T feature-major (1568-token streams).
  - Attention: S^T = k q'^T per head ([128+68 j-tiles, 196]); exp on Act;
    PV token-major with P^T as lhsT streaming only 65 cols (64 v-cols + a
    ones column that yields the softmax sums), so the softmax scale is a
    per-partition multiply: one strided reciprocal + one stride-0-broadcast
    tensor_mul per 6-head psum bank.  No broadcast/ones matmuls at all.
  - O (token-major) is transposed back on the PE via identity matmuls
    (out_free=tsz) for the projection.
  - Projection D is tiled 13x128 over tokens PACKED ACROSS BATCHES (DRAM
    rows are contiguous), each tile DMA-ing out in 1-2 per-batch pieces.
DMA: every instruction costs ~625ns on the serial HWDGE device, so x loads
as batch-pairs (one DMA per (bp, ti), strided [tsz,2,768] view) and xT as
nt0-chunk + remainder per kt (k groups start after chunk 0).
Scheduling: dedicated psum pools decouple the S->exp chain from the GEMM
pipeline (fill/s/po/ot = 2/2/2/2 banks); v(2..7), deferred O-transposes and
D token-tiles interleave between attention pairs (1-2 filler units per
pair, throttled at queue length 16 to keep inventory for the final
batches).  Engine split: exp + A/v copies on Act; qk copies, normalize,
O^T copies (2x mode), D bias-adds on DVE.

Infra notes: this container's walrus accepts only ONE attached semaphore
wait per instruction -- _install_wait_split() hoists extra waits onto
standalone EventSemaphore instructions.  Timing is the concourse TimelineSim
cost model (NTFF profiling unavailable under this axon client): 171050 ns
vs 205577 ns for the v1 kernel (-16.8%), rel err 3.4e-03 verified on HW.
"""
import os
import sys
import numpy as np
import ml_dtypes

sys.path.insert(0, "/opt/trn_rl_repo")

SIZE, N_TOK, DIM, HEADS, HEAD_DIM, BATCH = 14, 196, 768, 12, 64, 64
N_CORES = 8
B_PER_CORE = BATCH // N_CORES  # 8
NT2 = 2 * N_TOK  # 392
NTB = N_TOK * B_PER_CORE  # 1568
BF16 = ml_dtypes.bfloat16

TOK_TILES = [(0, 128), (128, 68)]  # token-dim partition tiles (196 = 128+68)

LAST_EXEC_NS = None
LAST_TRACE = None


def _grid_g(factors):
    idx = np.arange(SIZE * SIZE).reshape(SIZE, SIZE)
    A = np.zeros((N_TOK, N_TOK), dtype=np.float32)
    for di, dj in [(-1, 0), (1, 0), (0, -1), (0, 1)]:
        for i in range(SIZE):
            for j in range(SIZE):
                ii, jj = i + di, j + dj
                if 0 <= ii < SIZE and 0 <= jj < SIZE:
                    A[idx[i, j], idx[ii, jj]] = 1.0
    NN = A / (A.sum(axis=1, keepdims=True) + 1.0)
    C = np.eye(N_TOK, dtype=np.float32) / 2.0
    return factors[0] * C + factors[1] * NN


def _install_wait_split():
    """This container's walrus rejects >1 attached semaphore wait per
    instruction ("Too many sync wait commands").  Hoist excess waits onto
    standalone InstEventSemaphore instructions just before, on the same
    engine — engine queues are in-order, so semantics are identical."""
    import concourse.mybir as mybir
    import concourse.tile as tile
    from concourse.vector_clock import ScopedClock

    TC = tile.TileContext
    if getattr(TC, "_wait_split_patched", False):
        return
    LIMIT = 1

    def _split(tc, inst):
        si = inst.sync_info
        if (si is None or not si.on_wait or len(si.on_wait) <= LIMIT
                or inst.engine == mybir.EngineType.Unassigned):
            return
        waits = list(si.on_wait)
        extra, keep = waits[:-LIMIT], waits[-LIMIT:]
        for i, w in enumerate(extra):
            ev = mybir.InstEventSemaphore(
                name=f"{inst.name}-ws{i}", engine=inst.engine,
                sync_info=mybir.SyncInfo(on_wait=[w], on_update=[]),
            )
            tc._add_instruction(ev)
        inst.sync_info = mybir.SyncInfo(on_wait=keep,
                                        on_update=list(si.on_update))

    orig_commit = TC._commit_instruction

    def patched_commit(self, inst, lazy_reg_writes=True):
        _split(self, inst)
        return orig_commit(self, inst, lazy_reg_writes=lazy_reg_writes)

    TC._commit_instruction = patched_commit

    def patched_drain_and_barrier(self, tick_clock, wait_clock):
        nc = self.nc
        probe = mybir.InstNoOp(
            name=f"drain-probe-{nc.next_id()}", engine=mybir.EngineType.SP)
        wait_clock.add_sem_waits(
            probe, ScopedClock({None: tick_clock.global_clock}))
        pw = probe.sync_info.on_wait if probe.sync_info else []
        for i, w in enumerate(pw):
            ev = mybir.InstEventSemaphore(
                name=f"drainw-{nc.next_id()}-{i}", engine=mybir.EngineType.SP,
                sync_info=mybir.SyncInfo(on_wait=[w], on_update=[]),
            )
            self._add_instruction(ev)
        nc.sync.drain()
        nc.all_engine_barrier()
        assert self.sems is not None
        popped = nc._tile_sem_poison_stack.pop()
        assert popped is self._sem_poison
        nc.clear_and_free_semaphores(list(self.sems.allocated().values()))
        nc.all_engine_barrier()

    TC._drain_and_barrier = patched_drain_and_barrier
    TC._wait_split_patched = True


def _build_bass():
    import concourse.bass as bass
    import concourse.mybir as mybir
    import concourse.tile as tile

    _install_wait_split()

    f32 = mybir.dt.float32
    bf16 = mybir.dt.bfloat16
    AF = mybir.ActivationFunctionType

    nc = bass.Bass()

    x_d = nc.declare_dram_parameter("x", [B_PER_CORE, N_TOK, DIM], bf16, isOutput=False)
    xT_d = nc.declare_dram_parameter("xT", [DIM, NTB], bf16, isOutput=False)
    gT_d = nc.declare_dram_parameter("gT", [N_TOK, N_TOK], bf16, isOutput=False)
    wq_d = nc.declare_dram_parameter("wqT", [DIM, DIM], bf16, isOutput=False)
    wk_d = nc.declare_dram_parameter("wkT", [DIM, DIM], bf16, isOutput=False)
    wv_d = nc.declare_dram_parameter("wvT", [DIM, DIM], bf16, isOutput=False)
    wp_d = nc.declare_dram_parameter("wpT", [DIM, DIM], bf16, isOutput=False)
    bias_d = nc.declare_dram_parameter("bias", [DIM], f32, isOutput=False)
    idn_d = nc.declare_dram_parameter("idn", [128, 128], bf16, isOutput=False)
    out_d = nc.declare_dram_parameter(
        "out", [B_PER_CORE, N_TOK, DIM], f32, isOutput=True
    )

    with tile.TileContext(nc) as tc:
        with (
            tc.tile_pool(name="const", bufs=1) as const_p,
            tc.tile_pool(name="big", bufs=1) as big_p,
            tc.tile_pool(name="tok", bufs=12) as tok_p,   # x then o_tok
            tc.tile_pool(name="cp", bufs=10) as cp_p,
            tc.tile_pool(name="rsp", bufs=8) as rs_p,
            tc.tile_pool(name="yp", bufs=4) as y_p,
            tc.tile_pool(name="ps_big", bufs=2, space="PSUM") as ps_big,
            tc.tile_pool(name="ps_s", bufs=2, space="PSUM") as ps_s,
            tc.tile_pool(name="ps_po", bufs=2, space="PSUM") as ps_po,
            tc.tile_pool(name="ps_ot", bufs=2, space="PSUM") as ps_ot,
        ):
            # ---- input DMAs (k-GEMM inputs first so PE starts ASAP;
            #      xT in nt-column chunks so k groups start after chunk 0) ----
            def load_w(d, nm, tiles=None):
                ts = []
                for kt in range(6):
                    t = const_p.tile([128, DIM], bf16, name=f"{nm}{kt}")
                    if tiles is None:
                        nc.sync.dma_start(out=t, in_=d[kt * 128:(kt + 1) * 128, :])
                    ts.append(t)
                return ts

            g_sb = []
            for ti, (t0, tsz) in enumerate(TOK_TILES):
                t = const_p.tile([128, N_TOK], bf16, name=f"g{ti}")
                nc.sync.dma_start(out=t[:tsz], in_=gT_d[t0:t0 + tsz, :])
                g_sb.append(t)

            # x loaded as batch-pairs: one DMA per (bp, ti) into a
            # [tsz, 2, 768] view (HWDGE charges ~625ns per DMA instruction)
            xp_sb = [[None, None] for _ in range(B_PER_CORE // 2)]

            def load_x(bp):
                for ti, (t0, tsz) in enumerate(TOK_TILES):
                    t = tok_p.tile([128, 2 * DIM], bf16,
                                   name=f"x{bp}_{ti}", tag="tok")
                    nc.sync.dma_start(
                        out=t.rearrange("p (s c) -> p s c", s=2)[:tsz],
                        in_=x_d[2 * bp:2 * bp + 2, t0:t0 + tsz, :]
                        .rearrange("s p c -> p s c"))
                    xp_sb[bp][ti] = t

            load_x(0)

            wk_sb = load_w(wk_d, "wk", tiles=False)
            xT_sb = [const_p.tile([128, NTB], bf16, name=f"xT{kt}")
                     for kt in range(6)]
            for kt in range(6):
                nc.sync.dma_start(out=wk_sb[kt],
                                  in_=wk_d[kt * 128:(kt + 1) * 128, :])
                nc.sync.dma_start(
                    out=xT_sb[kt][:, 0:NT2],
                    in_=xT_d[kt * 128:(kt + 1) * 128, 0:NT2])
            for kt in range(6):
                nc.sync.dma_start(
                    out=xT_sb[kt][:, NT2:4 * NT2],
                    in_=xT_d[kt * 128:(kt + 1) * 128, NT2:4 * NT2])
            for bp in range(1, B_PER_CORE // 2):
                load_x(bp)

            wq_sb = load_w(wq_d, "wq")
            wv_sb = load_w(wv_d, "wv")
            wp_sb = load_w(wp_d, "wp")
            bias_sb = const_p.tile([128, DIM], f32, name="bias")
            nc.sync.dma_start(out=bias_sb,
                              in_=bias_d[None, :].broadcast_to([128, DIM]))
            idn_sb = const_p.tile([128, 128], bf16, name="idn")
            nc.sync.dma_start(out=idn_sb, in_=idn_d[:, :])

            # ---- persistent activations ----
            xg_sb = [big_p.tile([128, NTB], bf16, name=f"xg{k}")
                     for k in range(6)]
            qT_sb = [big_p.tile([128, NTB], bf16, name=f"qT{k}")
                     for k in range(6)]
            kT_sb = [big_p.tile([128, NTB], bf16, name=f"kT{k}")
                     for k in range(6)]
            oT_sb = [big_p.tile([128, NTB], bf16, name=f"oT{k}")
                     for k in range(6)]
            # v: token-major, 12 heads x 65 cols (col 64 of each = ones)
            v_sb = [
                [big_p.tile([128, 780], bf16, name=f"v{b}_{ti}") for ti in range(2)]
                for b in range(B_PER_CORE)
            ]
            o_tok = [[None, None] for _ in range(B_PER_CORE)]  # token-major O

            # ---- stage B-k: k^T = Wk @ x^T (feature-major) ----
            def k_group(mt, nt):
                ps = ps_big.tile([128, NT2], f32, tag="big", name="ps")
                for kt in range(6):
                    nc.tensor.matmul(
                        ps, wk_sb[kt][:, mt * 128:(mt + 1) * 128],
                        xT_sb[kt][:, nt * NT2:(nt + 1) * NT2],
                        start=(kt == 0), stop=(kt == 5),
                    )
                nc.vector.tensor_copy(kT_sb[mt][:, nt * NT2:(nt + 1) * NT2], ps)

            # ---- stage A: xg^T[d,i] = sum_j x[j,d] G_s^T[j,i] ----
            # two batches per psum tile: one copy per two iters so the
            # psum->sbuf handoff latency stays off the PE critical path
            def a_iter2(bp, kt):
                ps = ps_big.tile([128, NT2], f32, tag="big", name="ps")
                for sub in range(2):
                    for ti, (t0, tsz) in enumerate(TOK_TILES):
                        nc.tensor.matmul(
                            ps[:, sub * N_TOK:(sub + 1) * N_TOK],
                            xp_sb[bp][ti][:tsz,
                                          sub * DIM + kt * 128:
                                          sub * DIM + (kt + 1) * 128],
                            g_sb[ti][:tsz],
                            start=(ti == 0), stop=(ti == 1),
                        )
                dst = xg_sb[kt][:, 2 * bp * N_TOK:(2 * bp + 2) * N_TOK]
                if (bp + kt) % 2 == 0:
                    nc.scalar.activation(dst, ps, AF.Copy)
                else:
                    nc.vector.tensor_copy(dst, ps)

            # ---- stage B-q: q'^T = Wq @ xg^T ----
            def q_group(mt, nt):
                ps = ps_big.tile([128, NT2], f32, tag="big", name="ps")
                for kt in range(6):
                    nc.tensor.matmul(
                        ps, wq_sb[kt][:, mt * 128:(mt + 1) * 128],
                        xg_sb[kt][:, nt * NT2:(nt + 1) * NT2],
                        start=(kt == 0), stop=(kt == 5),
                    )
                nc.vector.tensor_copy(qT_sb[mt][:, nt * NT2:(nt + 1) * NT2], ps)

            # ---- stage B-v: v token-major with interleaved ones cols ----
            def v_unit(b, ti, nt):
                t0, tsz = TOK_TILES[ti]
                ps = ps_big.tile([128, NT2], f32, tag="big", name="ps")
                for kt in range(6):
                    nc.tensor.matmul(
                        ps[:tsz, :384],
                        xT_sb[kt][:, b * N_TOK + t0:b * N_TOK + t0 + tsz],
                        wv_sb[kt][:, nt * 384:(nt + 1) * 384],
                        start=(kt == 0), stop=(kt == 5),
                    )
                dst = v_sb[b][ti].rearrange("p (h c) -> p h c", h=12)
                nc.scalar.activation(
                    dst[:tsz, nt * 6:(nt + 1) * 6, 0:64],
                    ps[:tsz, :384].rearrange("p (h c) -> p h c", h=6),
                    AF.Copy)
                if nt == 0:
                    nc.vector.memset(dst[:tsz, :, 64:65], 1.0)

            # ---- stage C: attention per (batch, head-pair) ----
            # po bank (b, mi, half): [tszi, 390] = heads 6*half..6*half+5,
            # 65 cols each (col 64 = softmax sums).
            po_banks = {}

            def c_pair(b, p):
                c0 = b * N_TOK
                half, hh = p // 3, None
                if p % 3 == 0:
                    for mi, (m0, msz) in enumerate(TOK_TILES):
                        po_banks[(b, mi, half)] = ps_po.tile(
                            [128, 390], f32, tag="po", name=f"po{b}_{mi}_{half}")
                pTs = []
                for hi in range(2):
                    h = 2 * p + hi
                    hb = hi * 64
                    s_ps = ps_s.tile([128, NT2], f32, tag="s", name="s")
                    for ti, (t0, tsz) in enumerate(TOK_TILES):
                        nc.tensor.matmul(
                            s_ps[:tsz, ti * N_TOK:(ti + 1) * N_TOK],
                            kT_sb[p][hb:hb + 64, c0 + t0:c0 + t0 + tsz],
                            qT_sb[p][hb:hb + 64, c0:c0 + N_TOK],
                            start=True, stop=True,
                        )
                    pT = cp_p.tile([128, NT2], bf16, tag="pT")
                    nc.scalar.activation(pT, s_ps, AF.Exp)
                    pTs.append(pT)
                for hi in range(2):
                    h = 2 * p + hi
                    hh = h - 6 * half
                    pT = pTs[hi]
                    for mi, (m0, msz) in enumerate(TOK_TILES):
                        po = po_banks[(b, mi, half)]
                        for ti, (t0, tsz) in enumerate(TOK_TILES):
                            nc.tensor.matmul(
                                po[:msz, 65 * hh:65 * hh + 65],
                                pT[:tsz, ti * N_TOK + m0:ti * N_TOK + m0 + msz],
                                v_sb[b][ti][:tsz, 65 * h:65 * h + 65],
                                start=(ti == 0), stop=(ti == 1),
                            )
                if p % 3 == 2:
                    # normalize heads 6*half..6*half+5 into o_tok
                    for mi, (m0, msz) in enumerate(TOK_TILES):
                        if half == 0 and o_tok[b][mi] is None:
                            o_tok[b][mi] = tok_p.tile(
                                [128, DIM], bf16, name=f"o{b}_{mi}", tag="tok")
                        po = po_banks.pop((b, mi, half))
                        pv = po.rearrange("p (h c) -> p h c", h=6)
                        rs = rs_p.tile([128, 6], bf16, tag="rs")
                        with nc.allow_low_precision(reason="softmax recip"):
                            nc.vector.reciprocal(rs[:msz], pv[:msz, :, 64])
                            ov = o_tok[b][mi].rearrange(
                                "p (h c) -> p h c", h=12)
                            nc.vector.tensor_mul(
                                ov[:msz, 6 * half:6 * half + 6, :],
                                pv[:msz, :, 0:64],
                                rs[:msz, :, None].broadcast_to([msz, 6, 64]),
                            )
            # transpose a group of 2 o_tok column-tiles -> oT (feature-major)
            def t_group(b, g):
                c0 = b * N_TOK
                mi = g // 3
                m0, msz = TOK_TILES[mi]
                for j in range(2):
                    kt = (g % 3) * 2 + j
                    ot = ps_ot.tile([128, 128], bf16, tag="ot")
                    nc.tensor.transpose(
                        ot[:, :msz],
                        o_tok[b][mi][:msz, kt * 128:(kt + 1) * 128],
                        idn_sb[:msz, :msz],
                    )
                    nc.vector.tensor_copy(
                        oT_sb[kt][:, c0 + m0:c0 + m0 + msz], ot[:, :msz])

            # ---- stage D: y = O @ Wp^T + bias; DMA out ----
            # token tiles packed across batch boundaries (13 x 128 instead of
            # 8 x (128+68)): DRAM rows are contiguous over (b, t), so each
            # tile DMAs out in 1-2 per-batch pieces
            def d_unit(tt, ti, nt):
                t0 = tt * 128
                tsz = min(128, NTB - t0)
                ps = ps_big.tile([128, NT2], f32, tag="big", name="ps")
                for kt in range(6):
                    nc.tensor.matmul(
                        ps[:tsz, :384],
                        oT_sb[kt][:, t0:t0 + tsz],
                        wp_sb[kt][:, nt * 384:(nt + 1) * 384],
                        start=(kt == 0), stop=(kt == 5),
                    )
                y_sb = y_p.tile([128, 384], f32, tag="y", name="y_sb")
                nc.vector.tensor_add(
                    y_sb[:tsz], ps[:tsz, :384],
                    bias_sb[:tsz, nt * 384:(nt + 1) * 384])
                r0 = t0
                while r0 < t0 + tsz:
                    b = r0 // N_TOK
                    r1 = min((b + 1) * N_TOK, t0 + tsz)
                    nc.sync.dma_start(
                        out=out_d[b, r0 - b * N_TOK:r1 - b * N_TOK,
                                  nt * 384:(nt + 1) * 384],
                        in_=y_sb[r0 - t0:r1 - t0])
                    r0 = r1

            # ---- schedule ----
            for kt in range(6):
                a_iter2(0, kt)
            for nt in range(4):
                for mt in range(6):
                    k_group(mt, nt)
            for bp in range(1, B_PER_CORE // 2):
                for kt in range(6):
                    a_iter2(bp, kt)
            for nt in range(4):
                for mt in range(6):
                    q_group(mt, nt)
            for b in (0, 1):
                for ti in range(2):
                    for nt in range(2):
                        v_unit(b, ti, nt)

            # C with v(2..7), deferred transposes, and D(b) units
            # interleaved between pairs (2 fillers per pair).
            from collections import deque
            fillers = deque()
            for b in range(2, B_PER_CORE):
                for ti in range(2):
                    for nt in range(2):
                        fillers.append(("v", b, ti, nt))

            def pop_fill(n):
                for _ in range(n):
                    if not fillers:
                        return
                    kind, fb, i1, i2 = fillers.popleft()
                    if kind == "v":
                        v_unit(fb, i1, i2)
                    elif kind == "t":
                        t_group(fb, i1)
                    else:
                        d_unit(fb, i1, i2)

            for b in range(B_PER_CORE):
                for p in range(6):
                    c_pair(b, p)
                    pop_fill(2 if len(fillers) > 16 else 1)
                for g in range(6):
                    fillers.append(("t", b, g, 0))
                # D token-tiles whose last contributing batch is b
                for tt in range(13):
                    tsz = min(128, NTB - tt * 128)
                    if (tt * 128 + tsz - 1) // N_TOK == b:
                        for nt in range(2):
                            fillers.append(("d", tt, 0, nt))
            pop_fill(10**9)

    return nc


_CACHED_NC = None


def kernel(x, w_qkv, w_proj, b_proj, factors):
    global LAST_EXEC_NS, LAST_TRACE, _CACHED_NC
    from concourse.bass_utils import run_bass_kernel_spmd

    factors = np.asarray(factors, dtype=np.float32)
    scale = HEAD_DIM ** -0.5
    G_s = _grid_g(factors) * scale

    w_qkv = np.asarray(w_qkv, dtype=np.float32)
    in_common = {
        "gT": np.ascontiguousarray(G_s.T).astype(BF16),
        "wqT": np.ascontiguousarray(w_qkv[0:DIM, :].T).astype(BF16),
        "wkT": np.ascontiguousarray(w_qkv[DIM:2 * DIM, :].T).astype(BF16),
        "wvT": np.ascontiguousarray(w_qkv[2 * DIM:3 * DIM, :].T).astype(BF16),
        "wpT": np.ascontiguousarray(np.asarray(w_proj, dtype=np.float32).T).astype(BF16),
        "bias": np.asarray(b_proj, dtype=np.float32),
        "idn": np.eye(128, dtype=np.float32).astype(BF16),
    }
    x = np.asarray(x, dtype=np.float32).astype(BF16)
    in_maps = []
    for c in range(N_CORES):
        xc = x[c * B_PER_CORE:(c + 1) * B_PER_CORE]
        xTc = np.ascontiguousarray(
            xc.reshape(NTB, DIM).T)
        in_maps.append({"x": xc, "xT": xTc, **in_common})

    if _CACHED_NC is None:
        _CACHED_NC = _build_bass()
    nc = _CACHED_NC

    trace = bool(int(os.environ.get("KERNEL_TRACE", "0")))
    res = run_bass_kernel_spmd(nc, in_maps, core_ids=list(range(N_CORES)),
                               trace=trace)
    LAST_EXEC_NS = res.exec_time_ns
    if res.instructions_and_trace is not None:
        LAST_TRACE = res.instructions_and_trace[1]
    out = np.concatenate([res.results[c]["out"] for c in range(N_CORES)], axis=0)
    return out.astype(np.float32)


# revision 5
# speedup vs baseline: 1.0534x; 1.0114x over previous
"""Trainium2 Bass kernel for nn_Attention_33157147525297 (v2 pipeline).

Graph-mixed multi-head attention, B=64, N=196 tokens, D=768, H=12 heads.
Data-parallel over batch: 8 batches per NeuronCore x 8 cores.

Math (host side): G mixes the query index only, so
  softmax(G (q k^T s)) v  ==  softmax((G_s q) k^T) v,  G_s = scale*G,
and G_s q = (G_s x) Wq^T, so the graph mix collapses to xg = G_s @ x.

Structure (cost model charges out_free_size * 0.417ns/col per matmul,
independent of K/M fill -- minimize total streamed columns, ~337k here):
  - x^T is pre-transposed on HOST (layout prep only); stage A computes just
    xg^T = x^T G_s^T via lhsT=x (token-major), rhs=G_s^T, two batches per
    psum tile so the psum->sbuf handoff stays off the PE critical path.
  - k^T = Wk x^T and q'^T = Wq xg^T feature-major (1568-token streams).
  - Attention: S^T = k q'^T per head ([128+68 j-tiles, 196]); exp on Act;
    PV token-major with P^T as lhsT streaming only 65 cols (64 v-cols + a
    ones column that yields the softmax sums), so the softmax scale is a
    per-partition multiply: one strided reciprocal + one stride-0-broadcast
    tensor_mul per 6-head psum bank.  No broadcast/ones matmuls at all.
  - O (token-major) is transposed back on the PE via identity matmuls
    (out_free=tsz) for the projection.
  - Projection D is tiled 13x128 over tokens PACKED ACROSS BATCHES (DRAM
    rows are contiguous), each tile DMA-ing out in 1-2 per-batch pieces.
Scheduling: dedicated psum pools decouple the S->exp chain from the GEMM
pipeline (fill/s/po/ot = 2/2/2/2 banks); v(2..7), deferred O-transposes and
D token-tiles interleave between attention pairs (1-2 filler units per
pair, throttled at queue length 16 to keep inventory for the final
batches) so the PE stays fed while exp chains complete.  Engine split:
exp + A/v copies on Act (plus late-batch O^T copies, when exp is done);
qk copies, normalize, O^T copies, D bias-adds on DVE.

Infra notes: this container's walrus accepts only ONE attached semaphore
wait per instruction -- _install_wait_split() hoists extra waits onto
standalone EventSemaphore instructions.  Timing is the concourse TimelineSim
cost model (NTFF profiling unavailable under this axon client): 170706 ns
vs 205577 ns for the v1 kernel (-17.0%), rel err 3.4e-03 verified on HW.
"""
import os
import sys
import numpy as np
import ml_dtypes

sys.path.insert(0, "/opt/trn_rl_repo")

SIZE, N_TOK, DIM, HEADS, HEAD_DIM, BATCH = 14, 196, 768, 12, 64, 64
N_CORES = 8
B_PER_CORE = BATCH // N_CORES  # 8
NT2 = 2 * N_TOK  # 392
NTB = N_TOK * B_PER_CORE  # 1568
BF16 = ml_dtypes.bfloat16

TOK_TILES = [(0, 128), (128, 68)]  # token-dim partition tiles (196 = 128+68)

LAST_EXEC_NS = None
LAST_TRACE = None


def _grid_g(factors):
    idx = np.arange(SIZE * SIZE).reshape(SIZE, SIZE)
    A = np.zeros((N_TOK, N_TOK), dtype=np.float32)
    for di, dj in [(-1, 0), (1, 0), (0, -1), (0, 1)]:
        for i in range(SIZE):
            for j in range(SIZE):
                ii, jj = i + di, j + dj
                if 0 <= ii < SIZE and 0 <= jj < SIZE:
                    A[idx[i, j], idx[ii, jj]] = 1.0
    NN = A / (A.sum(axis=1, keepdims=True) + 1.0)
    C = np.eye(N_TOK, dtype=np.float32) / 2.0
    return factors[0] * C + factors[1] * NN


def _install_wait_split():
    """This container's walrus rejects >1 attached semaphore wait per
    instruction ("Too many sync wait commands").  Hoist excess waits onto
    standalone InstEventSemaphore instructions just before, on the same
    engine — engine queues are in-order, so semantics are identical."""
    import concourse.mybir as mybir
    import concourse.tile as tile
    from concourse.vector_clock import ScopedClock

    TC = tile.TileContext
    if getattr(TC, "_wait_split_patched", False):
        return
    LIMIT = 1

    def _split(tc, inst):
        si = inst.sync_info
        if (si is None or not si.on_wait or len(si.on_wait) <= LIMIT
                or inst.engine == mybir.EngineType.Unassigned):
            return
        waits = list(si.on_wait)
        extra, keep = waits[:-LIMIT], waits[-LIMIT:]
        for i, w in enumerate(extra):
            ev = mybir.InstEventSemaphore(
                name=f"{inst.name}-ws{i}", engine=inst.engine,
                sync_info=mybir.SyncInfo(on_wait=[w], on_update=[]),
            )
            tc._add_instruction(ev)
        inst.sync_info = mybir.SyncInfo(on_wait=keep,
                                        on_update=list(si.on_update))

    orig_commit = TC._commit_instruction

    def patched_commit(self, inst, lazy_reg_writes=True):
        _split(self, inst)
        return orig_commit(self, inst, lazy_reg_writes=lazy_reg_writes)

    TC._commit_instruction = patched_commit

    def patched_drain_and_barrier(self, tick_clock, wait_clock):
        nc = self.nc
        probe = mybir.InstNoOp(
            name=f"drain-probe-{nc.next_id()}", engine=mybir.EngineType.SP)
        wait_clock.add_sem_waits(
            probe, ScopedClock({None: tick_clock.global_clock}))
        pw = probe.sync_info.on_wait if probe.sync_info else []
        for i, w in enumerate(pw):
            ev = mybir.InstEventSemaphore(
                name=f"drainw-{nc.next_id()}-{i}", engine=mybir.EngineType.SP,
                sync_info=mybir.SyncInfo(on_wait=[w], on_update=[]),
            )
            self._add_instruction(ev)
        nc.sync.drain()
        nc.all_engine_barrier()
        assert self.sems is not None
        popped = nc._tile_sem_poison_stack.pop()
        assert popped is self._sem_poison
        nc.clear_and_free_semaphores(list(self.sems.allocated().values()))
        nc.all_engine_barrier()

    TC._drain_and_barrier = patched_drain_and_barrier
    TC._wait_split_patched = True


def _build_bass():
    import concourse.bass as bass
    import concourse.mybir as mybir
    import concourse.tile as tile

    _install_wait_split()

    f32 = mybir.dt.float32
    bf16 = mybir.dt.bfloat16
    AF = mybir.ActivationFunctionType

    nc = bass.Bass()

    x_d = nc.declare_dram_parameter("x", [B_PER_CORE, N_TOK, DIM], bf16, isOutput=False)
    xT_d = nc.declare_dram_parameter("xT", [DIM, NTB], bf16, isOutput=False)
    gT_d = nc.declare_dram_parameter("gT", [N_TOK, N_TOK], bf16, isOutput=False)
    wq_d = nc.declare_dram_parameter("wqT", [DIM, DIM], bf16, isOutput=False)
    wk_d = nc.declare_dram_parameter("wkT", [DIM, DIM], bf16, isOutput=False)
    wv_d = nc.declare_dram_parameter("wvT", [DIM, DIM], bf16, isOutput=False)
    wp_d = nc.declare_dram_parameter("wpT", [DIM, DIM], bf16, isOutput=False)
    bias_d = nc.declare_dram_parameter("bias", [DIM], f32, isOutput=False)
    idn_d = nc.declare_dram_parameter("idn", [128, 128], bf16, isOutput=False)
    out_d = nc.declare_dram_parameter(
        "out", [B_PER_CORE, N_TOK, DIM], f32, isOutput=True
    )

    with tile.TileContext(nc) as tc:
        with (
            tc.tile_pool(name="const", bufs=1) as const_p,
            tc.tile_pool(name="big", bufs=1) as big_p,
            tc.tile_pool(name="tok", bufs=12) as tok_p,   # x then o_tok
            tc.tile_pool(name="cp", bufs=10) as cp_p,
            tc.tile_pool(name="rsp", bufs=8) as rs_p,
            tc.tile_pool(name="yp", bufs=4) as y_p,
            tc.tile_pool(name="ps_big", bufs=2, space="PSUM") as ps_big,
            tc.tile_pool(name="ps_s", bufs=2, space="PSUM") as ps_s,
            tc.tile_pool(name="ps_po", bufs=2, space="PSUM") as ps_po,
            tc.tile_pool(name="ps_ot", bufs=2, space="PSUM") as ps_ot,
        ):
            # ---- input DMAs (k-GEMM inputs first so PE starts ASAP;
            #      xT in nt-column chunks so k groups start after chunk 0) ----
            def load_w(d, nm, tiles=None):
                ts = []
                for kt in range(6):
                    t = const_p.tile([128, DIM], bf16, name=f"{nm}{kt}")
                    if tiles is None:
                        nc.sync.dma_start(out=t, in_=d[kt * 128:(kt + 1) * 128, :])
                    ts.append(t)
                return ts

            g_sb = [const_p.tile([128, N_TOK], bf16, name=f"g{ti}")
                    for ti in range(2)]

            # x loaded as batch-pairs: one DMA per (bp, ti) into a
            # [tsz, 2, 768] view (HWDGE charges ~625ns per DMA instruction)
            xp_sb = [[None, None] for _ in range(B_PER_CORE // 2)]

            def load_x(bp):
                for ti, (t0, tsz) in enumerate(TOK_TILES):
                    t = tok_p.tile([128, 2 * DIM], bf16,
                                   name=f"x{bp}_{ti}", tag="tok")
                    nc.sync.dma_start(
                        out=t.rearrange("p (s c) -> p s c", s=2)[:tsz],
                        in_=x_d[2 * bp:2 * bp + 2, t0:t0 + tsz, :]
                        .rearrange("s p c -> p s c"))
                    xp_sb[bp][ti] = t

            # interleave g / x(bp0) tile DMAs so the first A matmul
            # (needs only g[0] + xp0[0]) is gated by two DMAs, not four
            nc.sync.dma_start(out=g_sb[0][:128], in_=gT_d[0:128, :])
            t = tok_p.tile([128, 2 * DIM], bf16, name="x0_0", tag="tok")
            nc.sync.dma_start(
        out=t.rearrange("p (s c) -> p s c", s=2)[:128],
        in_=x_d[0:2, 0:128, :].rearrange("s p c -> p s c"))
            xp_sb[0][0] = t
            nc.sync.dma_start(out=g_sb[1][:68], in_=gT_d[128:196, :])
            t = tok_p.tile([128, 2 * DIM], bf16, name="x0_1", tag="tok")
            nc.sync.dma_start(
        out=t.rearrange("p (s c) -> p s c", s=2)[:68],
        in_=x_d[0:2, 128:196, :].rearrange("s p c -> p s c"))
            xp_sb[0][1] = t

            wk_sb = load_w(wk_d, "wk", tiles=False)
            xT_sb = [const_p.tile([128, NTB], bf16, name=f"xT{kt}")
                     for kt in range(6)]
            for kt in range(6):
                nc.sync.dma_start(out=wk_sb[kt],
                                  in_=wk_d[kt * 128:(kt + 1) * 128, :])
                nc.sync.dma_start(
                    out=xT_sb[kt][:, 0:NT2],
                    in_=xT_d[kt * 128:(kt + 1) * 128, 0:NT2])
            for kt in range(6):
                nc.sync.dma_start(
                    out=xT_sb[kt][:, NT2:4 * NT2],
                    in_=xT_d[kt * 128:(kt + 1) * 128, NT2:4 * NT2])
            for bp in range(1, B_PER_CORE // 2):
                load_x(bp)

            wq_sb = load_w(wq_d, "wq")
            wv_sb = load_w(wv_d, "wv")
            wp_sb = load_w(wp_d, "wp")
            bias_sb = const_p.tile([128, DIM], f32, name="bias")
            nc.sync.dma_start(out=bias_sb,
                              in_=bias_d[None, :].broadcast_to([128, DIM]))
            idn_sb = const_p.tile([128, 128], bf16, name="idn")
            nc.sync.dma_start(out=idn_sb, in_=idn_d[:, :])

            # ---- persistent activations ----
            xg_sb = [big_p.tile([128, NTB], bf16, name=f"xg{k}")
                     for k in range(6)]
            qT_sb = [big_p.tile([128, NTB], bf16, name=f"qT{k}")
                     for k in range(6)]
            kT_sb = [big_p.tile([128, NTB], bf16, name=f"kT{k}")
                     for k in range(6)]
            oT_sb = [big_p.tile([128, NTB], bf16, name=f"oT{k}")
                     for k in range(6)]
            # v: token-major, 12 heads x 65 cols (col 64 of each = ones)
            v_sb = [
                [big_p.tile([128, 780], bf16, name=f"v{b}_{ti}") for ti in range(2)]
                for b in range(B_PER_CORE)
            ]
            o_tok = [[None, None] for _ in range(B_PER_CORE)]  # token-major O

            # ---- stage B-k: k^T = Wk @ x^T (feature-major) ----
            def k_group(mt, nt):
                ps = ps_big.tile([128, NT2], f32, tag="big", name="ps")
                for kt in range(6):
                    nc.tensor.matmul(
                        ps, wk_sb[kt][:, mt * 128:(mt + 1) * 128],
                        xT_sb[kt][:, nt * NT2:(nt + 1) * NT2],
                        start=(kt == 0), stop=(kt == 5),
                    )
                nc.vector.tensor_copy(kT_sb[mt][:, nt * NT2:(nt + 1) * NT2], ps)

            # ---- stage A: xg^T[d,i] = sum_j x[j,d] G_s^T[j,i] ----
            # two batches per psum tile: one copy per two iters so the
            # psum->sbuf handoff latency stays off the PE critical path
            def a_iter2(bp, kt):
                ps = ps_big.tile([128, NT2], f32, tag="big", name="ps")
                for sub in range(2):
                    for ti, (t0, tsz) in enumerate(TOK_TILES):
                        nc.tensor.matmul(
                            ps[:, sub * N_TOK:(sub + 1) * N_TOK],
                            xp_sb[bp][ti][:tsz,
                                          sub * DIM + kt * 128:
                                          sub * DIM + (kt + 1) * 128],
                            g_sb[ti][:tsz],
                            start=(ti == 0), stop=(ti == 1),
                        )
                dst = xg_sb[kt][:, 2 * bp * N_TOK:(2 * bp + 2) * N_TOK]
                if (bp + kt) % 2 == 0:
                    nc.scalar.activation(dst, ps, AF.Copy)
                else:
                    nc.vector.tensor_copy(dst, ps)

            # ---- stage B-q: q'^T = Wq @ xg^T ----
            def q_group(mt, nt):
                ps = ps_big.tile([128, NT2], f32, tag="big", name="ps")
                for kt in range(6):
                    nc.tensor.matmul(
                        ps, wq_sb[kt][:, mt * 128:(mt + 1) * 128],
                        xg_sb[kt][:, nt * NT2:(nt + 1) * NT2],
                        start=(kt == 0), stop=(kt == 5),
                    )
                nc.vector.tensor_copy(qT_sb[mt][:, nt * NT2:(nt + 1) * NT2], ps)

            # ---- stage B-v: v token-major with interleaved ones cols ----
            def v_unit(b, ti, nt):
                t0, tsz = TOK_TILES[ti]
                ps = ps_big.tile([128, NT2], f32, tag="big", name="ps")
                for kt in range(6):
                    nc.tensor.matmul(
                        ps[:tsz, :384],
                        xT_sb[kt][:, b * N_TOK + t0:b * N_TOK + t0 + tsz],
                        wv_sb[kt][:, nt * 384:(nt + 1) * 384],
                        start=(kt == 0), stop=(kt == 5),
                    )
                dst = v_sb[b][ti].rearrange("p (h c) -> p h c", h=12)
                nc.scalar.activation(
                    dst[:tsz, nt * 6:(nt + 1) * 6, 0:64],
                    ps[:tsz, :384].rearrange("p (h c) -> p h c", h=6),
                    AF.Copy)
                if nt == 0:
                    nc.vector.memset(dst[:tsz, :, 64:65], 1.0)

            # ---- stage C: attention per (batch, head-pair) ----
            # po bank (b, mi, half): [tszi, 390] = heads 6*half..6*half+5,
            # 65 cols each (col 64 = softmax sums).
            po_banks = {}

            def c_pair(b, p):
                c0 = b * N_TOK
                half, hh = p // 3, None
                if p % 3 == 0:
                    for mi, (m0, msz) in enumerate(TOK_TILES):
                        po_banks[(b, mi, half)] = ps_po.tile(
                            [128, 390], f32, tag="po", name=f"po{b}_{mi}_{half}")
                pTs = []
                for hi in range(2):
                    h = 2 * p + hi
                    hb = hi * 64
                    s_ps = ps_s.tile([128, NT2], f32, tag="s", name="s")
                    for ti, (t0, tsz) in enumerate(TOK_TILES):
                        nc.tensor.matmul(
                            s_ps[:tsz, ti * N_TOK:(ti + 1) * N_TOK],
                            kT_sb[p][hb:hb + 64, c0 + t0:c0 + t0 + tsz],
                            qT_sb[p][hb:hb + 64, c0:c0 + N_TOK],
                            start=True, stop=True,
                        )
                    pT = cp_p.tile([128, NT2], bf16, tag="pT")
                    nc.scalar.activation(pT, s_ps, AF.Exp)
                    pTs.append(pT)
                for hi in range(2):
                    h = 2 * p + hi
                    hh = h - 6 * half
                    pT = pTs[hi]
                    for mi, (m0, msz) in enumerate(TOK_TILES):
                        po = po_banks[(b, mi, half)]
                        for ti, (t0, tsz) in enumerate(TOK_TILES):
                            nc.tensor.matmul(
                                po[:msz, 65 * hh:65 * hh + 65],
                                pT[:tsz, ti * N_TOK + m0:ti * N_TOK + m0 + msz],
                                v_sb[b][ti][:tsz, 65 * h:65 * h + 65],
                                start=(ti == 0), stop=(ti == 1),
                            )
                if p % 3 == 2:
                    # normalize heads 6*half..6*half+5 into o_tok
                    for mi, (m0, msz) in enumerate(TOK_TILES):
                        if half == 0 and o_tok[b][mi] is None:
                            o_tok[b][mi] = tok_p.tile(
                                [128, DIM], bf16, name=f"o{b}_{mi}", tag="tok")
                        po = po_banks.pop((b, mi, half))
                        pv = po.rearrange("p (h c) -> p h c", h=6)
                        rs = rs_p.tile([128, 6], bf16, tag="rs")
                        with nc.allow_low_precision(reason="softmax recip"):
                            nc.vector.reciprocal(rs[:msz], pv[:msz, :, 64])
                            ov = o_tok[b][mi].rearrange(
                                "p (h c) -> p h c", h=12)
                            nc.vector.tensor_mul(
                                ov[:msz, 6 * half:6 * half + 6, :],
                                pv[:msz, :, 0:64],
                                rs[:msz, :, None].broadcast_to([msz, 6, 64]),
                            )
            # transpose a group of 2 o_tok column-tiles -> oT (feature-major)
            def t_group(b, g):
                c0 = b * N_TOK
                mi = g // 3
                m0, msz = TOK_TILES[mi]
                for j in range(2):
                    kt = (g % 3) * 2 + j
                    ot = ps_ot.tile([128, 128], bf16, tag="ot")
                    nc.tensor.transpose(
                        ot[:, :msz],
                        o_tok[b][mi][:msz, kt * 128:(kt + 1) * 128],
                        idn_sb[:msz, :msz],
                    )
                    if b >= 6:
                        nc.scalar.activation(
                            oT_sb[kt][:, c0 + m0:c0 + m0 + msz], ot[:, :msz],
                            AF.Copy)
                    else:
                        nc.vector.tensor_copy(
                            oT_sb[kt][:, c0 + m0:c0 + m0 + msz], ot[:, :msz])

            # ---- stage D: y = O @ Wp^T + bias; DMA out ----
            # token tiles packed across batch boundaries (13 x 128 instead of
            # 8 x (128+68)): DRAM rows are contiguous over (b, t), so each
            # tile DMAs out in 1-2 per-batch pieces
            def d_unit(tt, ti, nt):
                t0 = tt * 128
                tsz = min(128, NTB - t0)
                ps = ps_big.tile([128, NT2], f32, tag="big", name="ps")
                for kt in range(6):
                    nc.tensor.matmul(
                        ps[:tsz, :384],
                        oT_sb[kt][:, t0:t0 + tsz],
                        wp_sb[kt][:, nt * 384:(nt + 1) * 384],
                        start=(kt == 0), stop=(kt == 5),
                    )
                y_sb = y_p.tile([128, 384], f32, tag="y", name="y_sb")
                nc.vector.tensor_add(
                    y_sb[:tsz], ps[:tsz, :384],
                    bias_sb[:tsz, nt * 384:(nt + 1) * 384])
                r0 = t0
                while r0 < t0 + tsz:
                    b = r0 // N_TOK
                    r1 = min((b + 1) * N_TOK, t0 + tsz)
                    nc.sync.dma_start(
                        out=out_d[b, r0 - b * N_TOK:r1 - b * N_TOK,
                                  nt * 384:(nt + 1) * 384],
                        in_=y_sb[r0 - t0:r1 - t0])
                    r0 = r1

            # ---- schedule ----
            for kt in range(6):
                a_iter2(0, kt)
            for nt in range(4):
                for mt in range(6):
                    k_group(mt, nt)
            for bp in range(1, B_PER_CORE // 2):
                for kt in range(6):
                    a_iter2(bp, kt)
            for nt in range(4):
                for mt in range(6):
                    q_group(mt, nt)
            for b in (0, 1):
                for ti in range(2):
                    for nt in range(2):
                        v_unit(b, ti, nt)

            # C with v(2..7), deferred transposes, and D(b) units
            # interleaved between pairs (2 fillers per pair).
            from collections import deque
            fillers = deque()
            for b in range(2, B_PER_CORE):
                for ti in range(2):
                    for nt in range(2):
                        fillers.append(("v", b, ti, nt))

            def pop_fill(n):
                for _ in range(n):
                    if not fillers:
                        return
                    kind, fb, i1, i2 = fillers.popleft()
                    if kind == "v":
                        v_unit(fb, i1, i2)
                    elif kind == "t":
                        t_group(fb, i1)
                    else:
                        d_unit(fb, i1, i2)

            for b in range(B_PER_CORE):
                for p in range(6):
                    c_pair(b, p)
                    pop_fill(2 if len(fillers) > 16 else 1)
                for g in range(6):
                    fillers.append(("t", b, g, 0))
                # D token-tiles whose last contributing batch is b
                for tt in range(13):
                    tsz = min(128, NTB - tt * 128)
                    if (tt * 128 + tsz - 1) // N_TOK == b:
                        for nt in range(2):
                            fillers.append(("d", tt, 0, nt))
            pop_fill(10**9)

    return nc


_CACHED_NC = None


def kernel(x, w_qkv, w_proj, b_proj, factors):
    global LAST_EXEC_NS, LAST_TRACE, _CACHED_NC
    from concourse.bass_utils import run_bass_kernel_spmd

    factors = np.asarray(factors, dtype=np.float32)
    scale = HEAD_DIM ** -0.5
    G_s = _grid_g(factors) * scale

    w_qkv = np.asarray(w_qkv, dtype=np.float32)
    in_common = {
        "gT": np.ascontiguousarray(G_s.T).astype(BF16),
        "wqT": np.ascontiguousarray(w_qkv[0:DIM, :].T).astype(BF16),
        "wkT": np.ascontiguousarray(w_qkv[DIM:2 * DIM, :].T).astype(BF16),
        "wvT": np.ascontiguousarray(w_qkv[2 * DIM:3 * DIM, :].T).astype(BF16),
        "wpT": np.ascontiguousarray(np.asarray(w_proj, dtype=np.float32).T).astype(BF16),
        "bias": np.asarray(b_proj, dtype=np.float32),
        "idn": np.eye(128, dtype=np.float32).astype(BF16),
    }
    x = np.asarray(x, dtype=np.float32).astype(BF16)
    in_maps = []
    for c in range(N_CORES):
        xc = x[c * B_PER_CORE:(c + 1) * B_PER_CORE]
        xTc = np.ascontiguousarray(
            xc.reshape(NTB, DIM).T)
        in_maps.append({"x": xc, "xT": xTc, **in_common})

    if _CACHED_NC is None:
        _CACHED_NC = _build_bass()
    nc = _CACHED_NC

    trace = bool(int(os.environ.get("KERNEL_TRACE", "0")))
    res = run_bass_kernel_spmd(nc, in_maps, core_ids=list(range(N_CORES)),
                               trace=trace)
    LAST_EXEC_NS = res.exec_time_ns
    if res.instructions_and_trace is not None:
        LAST_TRACE = res.instructions_and_trace[1]
    out = np.concatenate([res.results[c]["out"] for c in range(N_CORES)], axis=0)
    return out.astype(np.float32)


# revision 7
# speedup vs baseline: 1.0658x; 1.0118x over previous
"""Trainium2 Bass kernel for nn_Attention_33157147525297 (v2 pipeline).

Graph-mixed multi-head attention, B=64, N=196 tokens, D=768, H=12 heads.
Data-parallel over batch: 8 batches per NeuronCore x 8 cores.

Math (host side): G mixes the query index only, so
  softmax(G (q k^T s)) v  ==  softmax((G_s q) k^T) v,  G_s = scale*G,
and G_s q = (G_s x) Wq^T, so the graph mix collapses to xg = G_s @ x.

Structure (cost model charges out_free_size * 0.417ns/col per matmul,
independent of K/M fill -- minimize total streamed columns, ~337k here):
  - x^T is pre-transposed on HOST (layout prep only); stage A computes just
    xg^T = x^T G_s^T via lhsT=x (token-major), rhs=G_s^T, two batches per
    psum tile so the psum->sbuf handoff stays off the PE critical path.
  - k^T = Wk x^T and q'^T = Wq xg^T feature-major (1568-token streams).
  - Attention: S^T = k q'^T per head ([128+68 j-tiles, 196]); exp on Act;
    PV token-major with P^T as lhsT streaming only 65 cols (64 v-cols + a
    ones column that yields the softmax sums), so the softmax scale is a
    per-partition multiply: one strided reciprocal + one stride-0-broadcast
    tensor_mul per 6-head psum bank.  No broadcast/ones matmuls at all.
  - O (token-major) is transposed back on the PE via identity matmuls
    (out_free=tsz) for the projection.
  - Projection D is tiled 13x128 over tokens PACKED ACROSS BATCHES (DRAM
    rows are contiguous), each tile DMA-ing out in 1-2 per-batch pieces.
Scheduling: dedicated psum pools decouple the S->exp chain from the GEMM
pipeline (fill/s/po/ot = 2/2/2/2 banks); the attention loop is SOFTWARE
PIPELINED one pair deep (pair n+1's S+exp issue before pair n's PVs, so
the S->exp chain is never queued behind fillers and Act stays saturated);
v(2..7), deferred O-transposes and D token-tiles interleave between pairs
(1-2 filler units per pair, throttled at queue length 16 to keep inventory
for the final batches).  Engine split: exp + A/v copies (+late-batch O^T
copies) on Act; qk copies, normalize, O^T copies, D bias-adds on DVE.

Infra notes: this container's walrus accepts only ONE attached semaphore
wait per instruction -- _install_wait_split() hoists extra waits onto
standalone EventSemaphore instructions.  Timing is the concourse TimelineSim
cost model (NTFF profiling unavailable under this axon client): 166818 ns
vs 205577 ns for the v1 kernel (-18.9%), rel err 3.4e-03 verified on HW.
"""
import os
import sys
import numpy as np
import ml_dtypes

sys.path.insert(0, "/opt/trn_rl_repo")

SIZE, N_TOK, DIM, HEADS, HEAD_DIM, BATCH = 14, 196, 768, 12, 64, 64
N_CORES = 8
B_PER_CORE = BATCH // N_CORES  # 8
NT2 = 2 * N_TOK  # 392
NTB = N_TOK * B_PER_CORE  # 1568
BF16 = ml_dtypes.bfloat16

TOK_TILES = [(0, 128), (128, 68)]  # token-dim partition tiles (196 = 128+68)

LAST_EXEC_NS = None
LAST_TRACE = None


def _grid_g(factors):
    idx = np.arange(SIZE * SIZE).reshape(SIZE, SIZE)
    A = np.zeros((N_TOK, N_TOK), dtype=np.float32)
    for di, dj in [(-1, 0), (1, 0), (0, -1), (0, 1)]:
        for i in range(SIZE):
            for j in range(SIZE):
                ii, jj = i + di, j + dj
                if 0 <= ii < SIZE and 0 <= jj < SIZE:
                    A[idx[i, j], idx[ii, jj]] = 1.0
    NN = A / (A.sum(axis=1, keepdims=True) + 1.0)
    C = np.eye(N_TOK, dtype=np.float32) / 2.0
    return factors[0] * C + factors[1] * NN


def _install_wait_split():
    """This container's walrus rejects >1 attached semaphore wait per
    instruction ("Too many sync wait commands").  Hoist excess waits onto
    standalone InstEventSemaphore instructions just before, on the same
    engine — engine queues are in-order, so semantics are identical."""
    import concourse.mybir as mybir
    import concourse.tile as tile
    from concourse.vector_clock import ScopedClock

    TC = tile.TileContext
    if getattr(TC, "_wait_split_patched", False):
        return
    LIMIT = 1

    def _split(tc, inst):
        si = inst.sync_info
        if (si is None or not si.on_wait or len(si.on_wait) <= LIMIT
                or inst.engine == mybir.EngineType.Unassigned):
            return
        waits = list(si.on_wait)
        extra, keep = waits[:-LIMIT], waits[-LIMIT:]
        for i, w in enumerate(extra):
            ev = mybir.InstEventSemaphore(
                name=f"{inst.name}-ws{i}", engine=inst.engine,
                sync_info=mybir.SyncInfo(on_wait=[w], on_update=[]),
            )
            tc._add_instruction(ev)
        inst.sync_info = mybir.SyncInfo(on_wait=keep,
                                        on_update=list(si.on_update))

    orig_commit = TC._commit_instruction

    def patched_commit(self, inst, lazy_reg_writes=True):
        _split(self, inst)
        return orig_commit(self, inst, lazy_reg_writes=lazy_reg_writes)

    TC._commit_instruction = patched_commit

    def patched_drain_and_barrier(self, tick_clock, wait_clock):
        nc = self.nc
        probe = mybir.InstNoOp(
            name=f"drain-probe-{nc.next_id()}", engine=mybir.EngineType.SP)
        wait_clock.add_sem_waits(
            probe, ScopedClock({None: tick_clock.global_clock}))
        pw = probe.sync_info.on_wait if probe.sync_info else []
        for i, w in enumerate(pw):
            ev = mybir.InstEventSemaphore(
                name=f"drainw-{nc.next_id()}-{i}", engine=mybir.EngineType.SP,
                sync_info=mybir.SyncInfo(on_wait=[w], on_update=[]),
            )
            self._add_instruction(ev)
        nc.sync.drain()
        nc.all_engine_barrier()
        assert self.sems is not None
        popped = nc._tile_sem_poison_stack.pop()
        assert popped is self._sem_poison
        nc.clear_and_free_semaphores(list(self.sems.allocated().values()))
        nc.all_engine_barrier()

    TC._drain_and_barrier = patched_drain_and_barrier
    TC._wait_split_patched = True


def _build_bass():
    import concourse.bass as bass
    import concourse.mybir as mybir
    import concourse.tile as tile

    _install_wait_split()

    f32 = mybir.dt.float32
    bf16 = mybir.dt.bfloat16
    AF = mybir.ActivationFunctionType

    nc = bass.Bass()

    x_d = nc.declare_dram_parameter("x", [B_PER_CORE, N_TOK, DIM], bf16, isOutput=False)
    xT_d = nc.declare_dram_parameter("xT", [DIM, NTB], bf16, isOutput=False)
    gT_d = nc.declare_dram_parameter("gT", [N_TOK, N_TOK], bf16, isOutput=False)
    wq_d = nc.declare_dram_parameter("wqT", [DIM, DIM], bf16, isOutput=False)
    wk_d = nc.declare_dram_parameter("wkT", [DIM, DIM], bf16, isOutput=False)
    wv_d = nc.declare_dram_parameter("wvT", [DIM, DIM], bf16, isOutput=False)
    wp_d = nc.declare_dram_parameter("wpT", [DIM, DIM], bf16, isOutput=False)
    bias_d = nc.declare_dram_parameter("bias", [DIM], f32, isOutput=False)
    idn_d = nc.declare_dram_parameter("idn", [128, 128], bf16, isOutput=False)
    out_d = nc.declare_dram_parameter(
        "out", [B_PER_CORE, N_TOK, DIM], f32, isOutput=True
    )

    with tile.TileContext(nc) as tc:
        with (
            tc.tile_pool(name="const", bufs=1) as const_p,
            tc.tile_pool(name="big", bufs=1) as big_p,
            tc.tile_pool(name="tok", bufs=12) as tok_p,   # x then o_tok
            tc.tile_pool(name="cp", bufs=10) as cp_p,
            tc.tile_pool(name="rsp", bufs=8) as rs_p,
            tc.tile_pool(name="yp", bufs=4) as y_p,
            tc.tile_pool(name="ps_big", bufs=2, space="PSUM") as ps_big,
            tc.tile_pool(name="ps_s", bufs=2, space="PSUM") as ps_s,
            tc.tile_pool(name="ps_po", bufs=2, space="PSUM") as ps_po,
            tc.tile_pool(name="ps_ot", bufs=2, space="PSUM") as ps_ot,
        ):
            # ---- input DMAs (k-GEMM inputs first so PE starts ASAP;
            #      xT in nt-column chunks so k groups start after chunk 0) ----
            def load_w(d, nm, tiles=None):
                ts = []
                for kt in range(6):
                    t = const_p.tile([128, DIM], bf16, name=f"{nm}{kt}")
                    if tiles is None:
                        nc.sync.dma_start(out=t, in_=d[kt * 128:(kt + 1) * 128, :])
                    ts.append(t)
                return ts

            g_sb = [const_p.tile([128, N_TOK], bf16, name=f"g{ti}")
                    for ti in range(2)]

            # x loaded as batch-pairs: one DMA per (bp, ti) into a
            # [tsz, 2, 768] view (HWDGE charges ~625ns per DMA instruction)
            xp_sb = [[None, None] for _ in range(B_PER_CORE // 2)]

            def load_x(bp):
                for ti, (t0, tsz) in enumerate(TOK_TILES):
                    t = tok_p.tile([128, 2 * DIM], bf16,
                                   name=f"x{bp}_{ti}", tag="tok")
                    nc.sync.dma_start(
                        out=t.rearrange("p (s c) -> p s c", s=2)[:tsz],
                        in_=x_d[2 * bp:2 * bp + 2, t0:t0 + tsz, :]
                        .rearrange("s p c -> p s c"))
                    xp_sb[bp][ti] = t

            # interleave g / x(bp0) tile DMAs so the first A matmul
            # (needs only g[0] + xp0[0]) is gated by two DMAs, not four
            nc.sync.dma_start(out=g_sb[0][:128], in_=gT_d[0:128, :])
            t = tok_p.tile([128, 2 * DIM], bf16, name="x0_0", tag="tok")
            nc.sync.dma_start(
        out=t.rearrange("p (s c) -> p s c", s=2)[:128],
        in_=x_d[0:2, 0:128, :].rearrange("s p c -> p s c"))
            xp_sb[0][0] = t
            nc.sync.dma_start(out=g_sb[1][:68], in_=gT_d[128:196, :])
            t = tok_p.tile([128, 2 * DIM], bf16, name="x0_1", tag="tok")
            nc.sync.dma_start(
        out=t.rearrange("p (s c) -> p s c", s=2)[:68],
        in_=x_d[0:2, 128:196, :].rearrange("s p c -> p s c"))
            xp_sb[0][1] = t

            load_x(1)
            load_x(2)
            load_x(3)
            wk_sb = load_w(wk_d, "wk", tiles=False)
            xT_sb = [const_p.tile([128, NTB], bf16, name=f"xT{kt}")
                     for kt in range(6)]
            for kt in range(6):
                nc.sync.dma_start(out=wk_sb[kt],
                                  in_=wk_d[kt * 128:(kt + 1) * 128, :])
                nc.sync.dma_start(
                    out=xT_sb[kt][:, 0:NT2],
                    in_=xT_d[kt * 128:(kt + 1) * 128, 0:NT2])
            for kt in range(6):
                nc.sync.dma_start(
                    out=xT_sb[kt][:, NT2:4 * NT2],
                    in_=xT_d[kt * 128:(kt + 1) * 128, NT2:4 * NT2])


            wq_sb = load_w(wq_d, "wq")
            wv_sb = load_w(wv_d, "wv")
            wp_sb = load_w(wp_d, "wp")
            bias_sb = const_p.tile([128, DIM], f32, name="bias")
            nc.sync.dma_start(out=bias_sb,
                              in_=bias_d[None, :].broadcast_to([128, DIM]))
            idn_sb = const_p.tile([128, 128], bf16, name="idn")
            nc.sync.dma_start(out=idn_sb, in_=idn_d[:, :])

            # ---- persistent activations ----
            xg_sb = [big_p.tile([128, NTB], bf16, name=f"xg{k}")
                     for k in range(6)]
            qT_sb = [big_p.tile([128, NTB], bf16, name=f"qT{k}")
                     for k in range(6)]
            kT_sb = [big_p.tile([128, NTB], bf16, name=f"kT{k}")
                     for k in range(6)]
            oT_sb = [big_p.tile([128, NTB], bf16, name=f"oT{k}")
                     for k in range(6)]
            # v: token-major, 12 heads x 65 cols (col 64 of each = ones)
            v_sb = [
                [big_p.tile([128, 780], bf16, name=f"v{b}_{ti}") for ti in range(2)]
                for b in range(B_PER_CORE)
            ]
            o_tok = [[None, None] for _ in range(B_PER_CORE)]  # token-major O

            # ---- stage B-k: k^T = Wk @ x^T (feature-major) ----
            def k_group(mt, nt):
                ps = ps_big.tile([128, NT2], f32, tag="big", name="ps")
                for kt in range(6):
                    nc.tensor.matmul(
                        ps, wk_sb[kt][:, mt * 128:(mt + 1) * 128],
                        xT_sb[kt][:, nt * NT2:(nt + 1) * NT2],
                        start=(kt == 0), stop=(kt == 5),
                    )
                nc.vector.tensor_copy(kT_sb[mt][:, nt * NT2:(nt + 1) * NT2], ps)

            # ---- stage A: xg^T[d,i] = sum_j x[j,d] G_s^T[j,i] ----
            # two batches per psum tile: one copy per two iters so the
            # psum->sbuf handoff latency stays off the PE critical path
            def a_iter2(bp, kt):
                ps = ps_big.tile([128, NT2], f32, tag="big", name="ps")
                for sub in range(2):
                    for ti, (t0, tsz) in enumerate(TOK_TILES):
                        nc.tensor.matmul(
                            ps[:, sub * N_TOK:(sub + 1) * N_TOK],
                            xp_sb[bp][ti][:tsz,
                                          sub * DIM + kt * 128:
                                          sub * DIM + (kt + 1) * 128],
                            g_sb[ti][:tsz],
                            start=(ti == 0), stop=(ti == 1),
                        )
                dst = xg_sb[kt][:, 2 * bp * N_TOK:(2 * bp + 2) * N_TOK]
                if (bp + kt) % 2 == 0:
                    nc.scalar.activation(dst, ps, AF.Copy)
                else:
                    nc.vector.tensor_copy(dst, ps)

            # ---- stage B-q: q'^T = Wq @ xg^T ----
            def q_group(mt, nt):
                ps = ps_big.tile([128, NT2], f32, tag="big", name="ps")
                for kt in range(6):
                    nc.tensor.matmul(
                        ps, wq_sb[kt][:, mt * 128:(mt + 1) * 128],
                        xg_sb[kt][:, nt * NT2:(nt + 1) * NT2],
                        start=(kt == 0), stop=(kt == 5),
                    )
                nc.vector.tensor_copy(qT_sb[mt][:, nt * NT2:(nt + 1) * NT2], ps)

            # ---- stage B-v: v token-major with interleaved ones cols ----
            def v_unit(b, ti, nt):
                t0, tsz = TOK_TILES[ti]
                ps = ps_big.tile([128, NT2], f32, tag="big", name="ps")
                for kt in range(6):
                    nc.tensor.matmul(
                        ps[:tsz, :384],
                        xT_sb[kt][:, b * N_TOK + t0:b * N_TOK + t0 + tsz],
                        wv_sb[kt][:, nt * 384:(nt + 1) * 384],
                        start=(kt == 0), stop=(kt == 5),
                    )
                dst = v_sb[b][ti].rearrange("p (h c) -> p h c", h=12)
                nc.scalar.activation(
                    dst[:tsz, nt * 6:(nt + 1) * 6, 0:64],
                    ps[:tsz, :384].rearrange("p (h c) -> p h c", h=6),
                    AF.Copy)
                if nt == 0:
                    nc.vector.memset(dst[:tsz, :, 64:65], 1.0)

            # ---- stage C: attention per (batch, head-pair) ----
            # po bank (b, mi, half): [tszi, 390] = heads 6*half..6*half+5,
            # 65 cols each (col 64 = softmax sums).
            po_banks = {}

            pT_store = {}

            # front half: S matmuls + exp for both heads of the pair.
            # Issued one pair AHEAD of the PV half so the S->exp chain is
            # never queued behind filler units (keeps Act saturated).
            def c_front(b, p):
                c0 = b * N_TOK
                half = p // 3
                if p % 3 == 0:
                    for mi, (m0, msz) in enumerate(TOK_TILES):
                        po_banks[(b, mi, half)] = ps_po.tile(
                            [128, 390], f32, tag="po", name=f"po{b}_{mi}_{half}")
                pTs = []
                for hi in range(2):
                    h = 2 * p + hi
                    hb = hi * 64
                    s_ps = ps_s.tile([128, NT2], f32, tag="s", name="s")
                    for ti, (t0, tsz) in enumerate(TOK_TILES):
                        nc.tensor.matmul(
                            s_ps[:tsz, ti * N_TOK:(ti + 1) * N_TOK],
                            kT_sb[p][hb:hb + 64, c0 + t0:c0 + t0 + tsz],
                            qT_sb[p][hb:hb + 64, c0:c0 + N_TOK],
                            start=True, stop=True,
                        )
                    pT = cp_p.tile([128, NT2], bf16, tag="pT")
                    nc.scalar.activation(pT, s_ps, AF.Exp)
                    pTs.append(pT)
                pT_store[(b, p)] = pTs

            # back half: PV matmuls + (at p%3==2) softmax normalize
            def c_back(b, p):
                c0 = b * N_TOK
                half = p // 3
                pTs = pT_store.pop((b, p))
                for hi in range(2):
                    h = 2 * p + hi
                    hh = h - 6 * half
                    pT = pTs[hi]
                    for mi, (m0, msz) in enumerate(TOK_TILES):
                        po = po_banks[(b, mi, half)]
                        for ti, (t0, tsz) in enumerate(TOK_TILES):
                            nc.tensor.matmul(
                                po[:msz, 65 * hh:65 * hh + 65],
                                pT[:tsz, ti * N_TOK + m0:ti * N_TOK + m0 + msz],
                                v_sb[b][ti][:tsz, 65 * h:65 * h + 65],
                                start=(ti == 0), stop=(ti == 1),
                            )
                if p % 3 == 2:
                    # normalize heads 6*half..6*half+5 into o_tok
                    for mi, (m0, msz) in enumerate(TOK_TILES):
                        if half == 0 and o_tok[b][mi] is None:
                            o_tok[b][mi] = tok_p.tile(
                                [128, DIM], bf16, name=f"o{b}_{mi}", tag="tok")
                        po = po_banks.pop((b, mi, half))
                        pv = po.rearrange("p (h c) -> p h c", h=6)
                        rs = rs_p.tile([128, 6], bf16, tag="rs")
                        with nc.allow_low_precision(reason="softmax recip"):
                            nc.vector.reciprocal(rs[:msz], pv[:msz, :, 64])
                            ov = o_tok[b][mi].rearrange(
                                "p (h c) -> p h c", h=12)
                            nc.vector.tensor_mul(
                                ov[:msz, 6 * half:6 * half + 6, :],
                                pv[:msz, :, 0:64],
                                rs[:msz, :, None].broadcast_to([msz, 6, 64]),
                            )
            # transpose a group of 2 o_tok column-tiles -> oT (feature-major)
            def t_group(b, g):
                c0 = b * N_TOK
                mi = g // 3
                m0, msz = TOK_TILES[mi]
                for j in range(2):
                    kt = (g % 3) * 2 + j
                    ot = ps_ot.tile([128, 128], bf16, tag="ot")
                    nc.tensor.transpose(
                        ot[:, :msz],
                        o_tok[b][mi][:msz, kt * 128:(kt + 1) * 128],
                        idn_sb[:msz, :msz],
                    )
                    if b >= 6:
                        nc.scalar.activation(
                            oT_sb[kt][:, c0 + m0:c0 + m0 + msz], ot[:, :msz],
                            AF.Copy)
                    else:
                        nc.vector.tensor_copy(
                            oT_sb[kt][:, c0 + m0:c0 + m0 + msz], ot[:, :msz])

            # ---- stage D: y = O @ Wp^T + bias; DMA out ----
            # token tiles packed across batch boundaries (13 x 128 instead of
            # 8 x (128+68)): DRAM rows are contiguous over (b, t), so each
            # tile DMAs out in 1-2 per-batch pieces
            def d_unit(tt, ti, nt):
                t0 = tt * 128
                tsz = min(128, NTB - t0)
                ps = ps_big.tile([128, NT2], f32, tag="big", name="ps")
                for kt in range(6):
                    nc.tensor.matmul(
                        ps[:tsz, :384],
                        oT_sb[kt][:, t0:t0 + tsz],
                        wp_sb[kt][:, nt * 384:(nt + 1) * 384],
                        start=(kt == 0), stop=(kt == 5),
                    )
                y_sb = y_p.tile([128, 384], f32, tag="y", name="y_sb")
                nc.vector.tensor_add(
                    y_sb[:tsz], ps[:tsz, :384],
                    bias_sb[:tsz, nt * 384:(nt + 1) * 384])
                r0 = t0
                while r0 < t0 + tsz:
                    b = r0 // N_TOK
                    r1 = min((b + 1) * N_TOK, t0 + tsz)
                    nc.sync.dma_start(
                        out=out_d[b, r0 - b * N_TOK:r1 - b * N_TOK,
                                  nt * 384:(nt + 1) * 384],
                        in_=y_sb[r0 - t0:r1 - t0])
                    r0 = r1

            # ---- schedule ----
            for kt in range(6):
                a_iter2(0, kt)
            for kt in range(6):
                a_iter2(1, kt)
            for kt in range(6):
                a_iter2(2, kt)
            for kt in range(6):
                a_iter2(3, kt)
            for nt in range(4):
                for mt in range(6):
                    k_group(mt, nt)

            for nt in range(4):
                for mt in range(6):
                    q_group(mt, nt)
            for b in (0, 1):
                for ti in range(2):
                    for nt in range(2):
                        v_unit(b, ti, nt)

            # C with v(2..7), deferred transposes, and D(b) units
            # interleaved between pairs (2 fillers per pair).
            from collections import deque
            fillers = deque()
            for b in range(2, B_PER_CORE):
                for ti in range(2):
                    for nt in range(2):
                        fillers.append(("v", b, ti, nt))

            def pop_fill(n):
                for _ in range(n):
                    if not fillers:
                        return
                    kind, fb, i1, i2 = fillers.popleft()
                    if kind == "v":
                        v_unit(fb, i1, i2)
                    elif kind == "t":
                        t_group(fb, i1)
                    else:
                        d_unit(fb, i1, i2)

            seq = [(b, p) for b in range(B_PER_CORE) for p in range(6)]
            c_front(*seq[0])
            for i, (b, p) in enumerate(seq):
                if i + 1 < len(seq):
                    c_front(*seq[i + 1])
                c_back(b, p)
                if p == 5:
                    for g in range(6):
                        fillers.append(("t", b, g, 0))
                    # D token-tiles whose last contributing batch is b
                    for tt in range(13):
                        tsz = min(128, NTB - tt * 128)
                        if (tt * 128 + tsz - 1) // N_TOK == b:
                            for nt in range(2):
                                fillers.append(("d", tt, 0, nt))
                pop_fill(2 if len(fillers) > 16 else 1)
            pop_fill(10**9)

    return nc


_CACHED_NC = None


def kernel(x, w_qkv, w_proj, b_proj, factors):
    global LAST_EXEC_NS, LAST_TRACE, _CACHED_NC
    from concourse.bass_utils import run_bass_kernel_spmd

    factors = np.asarray(factors, dtype=np.float32)
    scale = HEAD_DIM ** -0.5
    G_s = _grid_g(factors) * scale

    w_qkv = np.asarray(w_qkv, dtype=np.float32)
    in_common = {
        "gT": np.ascontiguousarray(G_s.T).astype(BF16),
        "wqT": np.ascontiguousarray(w_qkv[0:DIM, :].T).astype(BF16),
        "wkT": np.ascontiguousarray(w_qkv[DIM:2 * DIM, :].T).astype(BF16),
        "wvT": np.ascontiguousarray(w_qkv[2 * DIM:3 * DIM, :].T).astype(BF16),
        "wpT": np.ascontiguousarray(np.asarray(w_proj, dtype=np.float32).T).astype(BF16),
        "bias": np.asarray(b_proj, dtype=np.float32),
        "idn": np.eye(128, dtype=np.float32).astype(BF16),
    }
    x = np.asarray(x, dtype=np.float32).astype(BF16)
    in_maps = []
    for c in range(N_CORES):
        xc = x[c * B_PER_CORE:(c + 1) * B_PER_CORE]
        xTc = np.ascontiguousarray(
            xc.reshape(NTB, DIM).T)
        in_maps.append({"x": xc, "xT": xTc, **in_common})

    if _CACHED_NC is None:
        _CACHED_NC = _build_bass()
    nc = _CACHED_NC

    trace = bool(int(os.environ.get("KERNEL_TRACE", "0")))
    res = run_bass_kernel_spmd(nc, in_maps, core_ids=list(range(N_CORES)),
                               trace=trace)
    LAST_EXEC_NS = res.exec_time_ns
    if res.instructions_and_trace is not None:
        LAST_TRACE = res.instructions_and_trace[1]
    out = np.concatenate([res.results[c]["out"] for c in range(N_CORES)], axis=0)
    return out.astype(np.float32)


# revision 8
# speedup vs baseline: 1.0661x; 1.0003x over previous
"""Trainium2 Bass kernel for nn_Attention_33157147525297 (v2 pipeline).

Graph-mixed multi-head attention, B=64, N=196 tokens, D=768, H=12 heads.
Data-parallel over batch: 8 batches per NeuronCore x 8 cores.

Math (host side): G mixes the query index only, so
  softmax(G (q k^T s)) v  ==  softmax((G_s q) k^T) v,  G_s = scale*G,
and G_s q = (G_s x) Wq^T, so the graph mix collapses to xg = G_s @ x.

Structure (cost model charges out_free_size * 0.417ns/col per matmul,
independent of K/M fill -- minimize total streamed columns, ~337k here):
  - x^T is pre-transposed on HOST (layout prep only); stage A computes just
    xg^T = x^T G_s^T via lhsT=x (token-major), rhs=G_s^T, two batches per
    psum tile so the psum->sbuf handoff stays off the PE critical path.
  - k^T = Wk x^T and q'^T = Wq xg^T feature-major (1568-token streams).
  - Attention: S^T = k q'^T per head ([128+68 j-tiles, 196]); exp on Act;
    PV token-major with P^T as lhsT streaming only 65 cols (64 v-cols + a
    ones column that yields the softmax sums), so the softmax scale is a
    per-partition multiply: one strided reciprocal + one stride-0-broadcast
    tensor_mul per 6-head psum bank.  No broadcast/ones matmuls at all.
  - O (token-major) is transposed back on the PE via identity matmuls
    (out_free=tsz) for the projection.
  - Projection D is tiled 13x128 over tokens PACKED ACROSS BATCHES (DRAM
    rows are contiguous), each tile DMA-ing out in 1-2 per-batch pieces.
Scheduling: dedicated psum pools decouple the S->exp chain from the GEMM
pipeline (fill/s/po/ot = 2/2/2/2 banks); v(2..7), deferred O-transposes and
D token-tiles interleave between attention pairs (1-2 filler units per
pair, throttled at queue length 16) so the PE stays fed while exp chains
complete; the attention loop is SOFTWARE PIPELINED one pair deep (pair
n+1's S+exp issue before pair n's PVs so the S->exp chain is never queued
behind fillers).  Engine split: exp + A copies (+late-batch O^T copies) on
Act; qk/v copies, normalize, O^T copies, D bias-adds on DVE.

Infra notes: this container's walrus accepts only ONE attached semaphore
wait per instruction -- _install_wait_split() hoists extra waits onto
standalone EventSemaphore instructions.  Timing is the concourse TimelineSim
cost model (NTFF profiling unavailable under this axon client): 166761 ns
vs 205577 ns for the v1 kernel (-18.9%), rel err 3.4e-03 verified on HW.
"""
import os
import sys
import numpy as np
import ml_dtypes

sys.path.insert(0, "/opt/trn_rl_repo")

SIZE, N_TOK, DIM, HEADS, HEAD_DIM, BATCH = 14, 196, 768, 12, 64, 64
N_CORES = 8
B_PER_CORE = BATCH // N_CORES  # 8
NT2 = 2 * N_TOK  # 392
NTB = N_TOK * B_PER_CORE  # 1568
BF16 = ml_dtypes.bfloat16

TOK_TILES = [(0, 128), (128, 68)]  # token-dim partition tiles (196 = 128+68)

LAST_EXEC_NS = None
LAST_TRACE = None


def _grid_g(factors):
    idx = np.arange(SIZE * SIZE).reshape(SIZE, SIZE)
    A = np.zeros((N_TOK, N_TOK), dtype=np.float32)
    for di, dj in [(-1, 0), (1, 0), (0, -1), (0, 1)]:
        for i in range(SIZE):
            for j in range(SIZE):
                ii, jj = i + di, j + dj
                if 0 <= ii < SIZE and 0 <= jj < SIZE:
                    A[idx[i, j], idx[ii, jj]] = 1.0
    NN = A / (A.sum(axis=1, keepdims=True) + 1.0)
    C = np.eye(N_TOK, dtype=np.float32) / 2.0
    return factors[0] * C + factors[1] * NN


def _install_wait_split():
    """This container's walrus rejects >1 attached semaphore wait per
    instruction ("Too many sync wait commands").  Hoist excess waits onto
    standalone InstEventSemaphore instructions just before, on the same
    engine — engine queues are in-order, so semantics are identical."""
    import concourse.mybir as mybir
    import concourse.tile as tile
    from concourse.vector_clock import ScopedClock

    TC = tile.TileContext
    if getattr(TC, "_wait_split_patched", False):
        return
    LIMIT = 1

    def _split(tc, inst):
        si = inst.sync_info
        if (si is None or not si.on_wait or len(si.on_wait) <= LIMIT
                or inst.engine == mybir.EngineType.Unassigned):
            return
        waits = list(si.on_wait)
        extra, keep = waits[:-LIMIT], waits[-LIMIT:]
        for i, w in enumerate(extra):
            ev = mybir.InstEventSemaphore(
                name=f"{inst.name}-ws{i}", engine=inst.engine,
                sync_info=mybir.SyncInfo(on_wait=[w], on_update=[]),
            )
            tc._add_instruction(ev)
        inst.sync_info = mybir.SyncInfo(on_wait=keep,
                                        on_update=list(si.on_update))

    orig_commit = TC._commit_instruction

    def patched_commit(self, inst, lazy_reg_writes=True):
        _split(self, inst)
        return orig_commit(self, inst, lazy_reg_writes=lazy_reg_writes)

    TC._commit_instruction = patched_commit

    def patched_drain_and_barrier(self, tick_clock, wait_clock):
        nc = self.nc
        probe = mybir.InstNoOp(
            name=f"drain-probe-{nc.next_id()}", engine=mybir.EngineType.SP)
        wait_clock.add_sem_waits(
            probe, ScopedClock({None: tick_clock.global_clock}))
        pw = probe.sync_info.on_wait if probe.sync_info else []
        for i, w in enumerate(pw):
            ev = mybir.InstEventSemaphore(
                name=f"drainw-{nc.next_id()}-{i}", engine=mybir.EngineType.SP,
                sync_info=mybir.SyncInfo(on_wait=[w], on_update=[]),
            )
            self._add_instruction(ev)
        nc.sync.drain()
        nc.all_engine_barrier()
        assert self.sems is not None
        popped = nc._tile_sem_poison_stack.pop()
        assert popped is self._sem_poison
        nc.clear_and_free_semaphores(list(self.sems.allocated().values()))
        nc.all_engine_barrier()

    TC._drain_and_barrier = patched_drain_and_barrier
    TC._wait_split_patched = True


def _build_bass():
    import concourse.bass as bass
    import concourse.mybir as mybir
    import concourse.tile as tile

    _install_wait_split()

    f32 = mybir.dt.float32
    bf16 = mybir.dt.bfloat16
    AF = mybir.ActivationFunctionType

    nc = bass.Bass()

    x_d = nc.declare_dram_parameter("x", [B_PER_CORE, N_TOK, DIM], bf16, isOutput=False)
    xT_d = nc.declare_dram_parameter("xT", [DIM, NTB], bf16, isOutput=False)
    gT_d = nc.declare_dram_parameter("gT", [N_TOK, N_TOK], bf16, isOutput=False)
    wq_d = nc.declare_dram_parameter("wqT", [DIM, DIM], bf16, isOutput=False)
    wk_d = nc.declare_dram_parameter("wkT", [DIM, DIM], bf16, isOutput=False)
    wv_d = nc.declare_dram_parameter("wvT", [DIM, DIM], bf16, isOutput=False)
    wp_d = nc.declare_dram_parameter("wpT", [DIM, DIM], bf16, isOutput=False)
    bias_d = nc.declare_dram_parameter("bias", [DIM], f32, isOutput=False)
    idn_d = nc.declare_dram_parameter("idn", [128, 128], bf16, isOutput=False)
    out_d = nc.declare_dram_parameter(
        "out", [B_PER_CORE, N_TOK, DIM], f32, isOutput=True
    )

    with tile.TileContext(nc) as tc:
        with (
            tc.tile_pool(name="const", bufs=1) as const_p,
            tc.tile_pool(name="big", bufs=1) as big_p,
            tc.tile_pool(name="tok", bufs=12) as tok_p,   # x then o_tok
            tc.tile_pool(name="cp", bufs=10) as cp_p,
            tc.tile_pool(name="rsp", bufs=8) as rs_p,
            tc.tile_pool(name="yp", bufs=4) as y_p,
            tc.tile_pool(name="ps_big", bufs=2, space="PSUM") as ps_big,
            tc.tile_pool(name="ps_s", bufs=2, space="PSUM") as ps_s,
            tc.tile_pool(name="ps_po", bufs=2, space="PSUM") as ps_po,
            tc.tile_pool(name="ps_ot", bufs=2, space="PSUM") as ps_ot,
        ):
            # ---- input DMAs (k-GEMM inputs first so PE starts ASAP;
            #      xT in nt-column chunks so k groups start after chunk 0) ----
            def load_w(d, nm, tiles=None):
                ts = []
                for kt in range(6):
                    t = const_p.tile([128, DIM], bf16, name=f"{nm}{kt}")
                    if tiles is None:
                        nc.sync.dma_start(out=t, in_=d[kt * 128:(kt + 1) * 128, :])
                    ts.append(t)
                return ts

            g_sb = [const_p.tile([128, N_TOK], bf16, name=f"g{ti}")
                    for ti in range(2)]

            # x loaded as batch-pairs: one DMA per (bp, ti) into a
            # [tsz, 2, 768] view (HWDGE charges ~625ns per DMA instruction)
            xp_sb = [[None, None] for _ in range(B_PER_CORE // 2)]

            def load_x(bp):
                for ti, (t0, tsz) in enumerate(TOK_TILES):
                    t = tok_p.tile([128, 2 * DIM], bf16,
                                   name=f"x{bp}_{ti}", tag="tok")
                    nc.sync.dma_start(
                        out=t.rearrange("p (s c) -> p s c", s=2)[:tsz],
                        in_=x_d[2 * bp:2 * bp + 2, t0:t0 + tsz, :]
                        .rearrange("s p c -> p s c"))
                    xp_sb[bp][ti] = t

            # interleave g / x(bp0) tile DMAs so the first A matmul
            # (needs only g[0] + xp0[0]) is gated by two DMAs, not four
            nc.sync.dma_start(out=g_sb[0][:128], in_=gT_d[0:128, :])
            t = tok_p.tile([128, 2 * DIM], bf16, name="x0_0", tag="tok")
            nc.sync.dma_start(
        out=t.rearrange("p (s c) -> p s c", s=2)[:128],
        in_=x_d[0:2, 0:128, :].rearrange("s p c -> p s c"))
            xp_sb[0][0] = t
            nc.sync.dma_start(out=g_sb[1][:68], in_=gT_d[128:196, :])
            t = tok_p.tile([128, 2 * DIM], bf16, name="x0_1", tag="tok")
            nc.sync.dma_start(
        out=t.rearrange("p (s c) -> p s c", s=2)[:68],
        in_=x_d[0:2, 128:196, :].rearrange("s p c -> p s c"))
            xp_sb[0][1] = t

            load_x(1)
            load_x(2)
            load_x(3)
            wk_sb = load_w(wk_d, "wk", tiles=False)
            xT_sb = [const_p.tile([128, NTB], bf16, name=f"xT{kt}")
                     for kt in range(6)]
            for kt in range(6):
                nc.sync.dma_start(out=wk_sb[kt],
                                  in_=wk_d[kt * 128:(kt + 1) * 128, :])
                nc.sync.dma_start(
                    out=xT_sb[kt][:, 0:NT2],
                    in_=xT_d[kt * 128:(kt + 1) * 128, 0:NT2])
            for kt in range(6):
                nc.sync.dma_start(
                    out=xT_sb[kt][:, NT2:4 * NT2],
                    in_=xT_d[kt * 128:(kt + 1) * 128, NT2:4 * NT2])


            wq_sb = load_w(wq_d, "wq")
            wv_sb = load_w(wv_d, "wv")
            wp_sb = load_w(wp_d, "wp")
            bias_sb = const_p.tile([128, DIM], f32, name="bias")
            nc.sync.dma_start(out=bias_sb,
                              in_=bias_d[None, :].broadcast_to([128, DIM]))
            idn_sb = const_p.tile([128, 128], bf16, name="idn")
            nc.sync.dma_start(out=idn_sb, in_=idn_d[:, :])

            # ---- persistent activations ----
            xg_sb = [big_p.tile([128, NTB], bf16, name=f"xg{k}")
                     for k in range(6)]
            qT_sb = [big_p.tile([128, NTB], bf16, name=f"qT{k}")
                     for k in range(6)]
            kT_sb = [big_p.tile([128, NTB], bf16, name=f"kT{k}")
                     for k in range(6)]
            oT_sb = [big_p.tile([128, NTB], bf16, name=f"oT{k}")
                     for k in range(6)]
            # v: token-major, 12 heads x 65 cols (col 64 of each = ones)
            v_sb = [
                [big_p.tile([128, 780], bf16, name=f"v{b}_{ti}") for ti in range(2)]
                for b in range(B_PER_CORE)
            ]
            o_tok = [[None, None] for _ in range(B_PER_CORE)]  # token-major O

            # ---- stage B-k: k^T = Wk @ x^T (feature-major) ----
            def k_group(mt, nt):
                ps = ps_big.tile([128, NT2], f32, tag="big", name="ps")
                for kt in range(6):
                    nc.tensor.matmul(
                        ps, wk_sb[kt][:, mt * 128:(mt + 1) * 128],
                        xT_sb[kt][:, nt * NT2:(nt + 1) * NT2],
                        start=(kt == 0), stop=(kt == 5),
                    )
                nc.vector.tensor_copy(kT_sb[mt][:, nt * NT2:(nt + 1) * NT2], ps)

            # ---- stage A: xg^T[d,i] = sum_j x[j,d] G_s^T[j,i] ----
            # two batches per psum tile: one copy per two iters so the
            # psum->sbuf handoff latency stays off the PE critical path
            def a_iter2(bp, kt):
                ps = ps_big.tile([128, NT2], f32, tag="big", name="ps")
                for sub in range(2):
                    for ti, (t0, tsz) in enumerate(TOK_TILES):
                        nc.tensor.matmul(
                            ps[:, sub * N_TOK:(sub + 1) * N_TOK],
                            xp_sb[bp][ti][:tsz,
                                          sub * DIM + kt * 128:
                                          sub * DIM + (kt + 1) * 128],
                            g_sb[ti][:tsz],
                            start=(ti == 0), stop=(ti == 1),
                        )
                dst = xg_sb[kt][:, 2 * bp * N_TOK:(2 * bp + 2) * N_TOK]
                if (bp + kt) % 2 == 0:
                    nc.scalar.activation(dst, ps, AF.Copy)
                else:
                    nc.vector.tensor_copy(dst, ps)

            # ---- stage B-q: q'^T = Wq @ xg^T ----
            def q_group(mt, nt):
                ps = ps_big.tile([128, NT2], f32, tag="big", name="ps")
                for kt in range(6):
                    nc.tensor.matmul(
                        ps, wq_sb[kt][:, mt * 128:(mt + 1) * 128],
                        xg_sb[kt][:, nt * NT2:(nt + 1) * NT2],
                        start=(kt == 0), stop=(kt == 5),
                    )
                nc.vector.tensor_copy(qT_sb[mt][:, nt * NT2:(nt + 1) * NT2], ps)

            # ---- stage B-v: v token-major with interleaved ones cols ----
            def v_unit(b, ti, nt):
                t0, tsz = TOK_TILES[ti]
                ps = ps_big.tile([128, NT2], f32, tag="big", name="ps")
                for kt in range(6):
                    nc.tensor.matmul(
                        ps[:tsz, :384],
                        xT_sb[kt][:, b * N_TOK + t0:b * N_TOK + t0 + tsz],
                        wv_sb[kt][:, nt * 384:(nt + 1) * 384],
                        start=(kt == 0), stop=(kt == 5),
                    )
                dst = v_sb[b][ti].rearrange("p (h c) -> p h c", h=12)
                nc.vector.tensor_copy(
                    dst[:tsz, nt * 6:(nt + 1) * 6, 0:64],
                    ps[:tsz, :384].rearrange("p (h c) -> p h c", h=6))
                if nt == 0:
                    nc.vector.memset(dst[:tsz, :, 64:65], 1.0)

            # ---- stage C: attention per (batch, head-pair) ----
            # po bank (b, mi, half): [tszi, 390] = heads 6*half..6*half+5,
            # 65 cols each (col 64 = softmax sums).
            po_banks = {}

            pT_store = {}

            # front half: S matmuls + exp for both heads of the pair.
            # Issued one pair AHEAD of the PV half so the S->exp chain is
            # never queued behind filler units (keeps Act saturated).
            def c_front(b, p):
                c0 = b * N_TOK
                half = p // 3
                if p % 3 == 0:
                    for mi, (m0, msz) in enumerate(TOK_TILES):
                        po_banks[(b, mi, half)] = ps_po.tile(
                            [128, 390], f32, tag="po", name=f"po{b}_{mi}_{half}")
                pTs = []
                for hi in range(2):
                    h = 2 * p + hi
                    hb = hi * 64
                    s_ps = ps_s.tile([128, NT2], f32, tag="s", name="s")
                    for ti, (t0, tsz) in enumerate(TOK_TILES):
                        nc.tensor.matmul(
                            s_ps[:tsz, ti * N_TOK:(ti + 1) * N_TOK],
                            kT_sb[p][hb:hb + 64, c0 + t0:c0 + t0 + tsz],
                            qT_sb[p][hb:hb + 64, c0:c0 + N_TOK],
                            start=True, stop=True,
                        )
                    pT = cp_p.tile([128, NT2], bf16, tag="pT")
                    nc.scalar.activation(pT, s_ps, AF.Exp)
                    pTs.append(pT)
                pT_store[(b, p)] = pTs

            # back half: PV matmuls + (at p%3==2) softmax normalize
            def c_back(b, p):
                c0 = b * N_TOK
                half = p // 3
                pTs = pT_store.pop((b, p))
                for hi in range(2):
                    h = 2 * p + hi
                    hh = h - 6 * half
                    pT = pTs[hi]
                    for mi, (m0, msz) in enumerate(TOK_TILES):
                        po = po_banks[(b, mi, half)]
                        for ti, (t0, tsz) in enumerate(TOK_TILES):
                            nc.tensor.matmul(
                                po[:msz, 65 * hh:65 * hh + 65],
                                pT[:tsz, ti * N_TOK + m0:ti * N_TOK + m0 + msz],
                                v_sb[b][ti][:tsz, 65 * h:65 * h + 65],
                                start=(ti == 0), stop=(ti == 1),
                            )
                if p % 3 == 2:
                    # normalize heads 6*half..6*half+5 into o_tok
                    for mi, (m0, msz) in enumerate(TOK_TILES):
                        if half == 0 and o_tok[b][mi] is None:
                            o_tok[b][mi] = tok_p.tile(
                                [128, DIM], bf16, name=f"o{b}_{mi}", tag="tok")
                        po = po_banks.pop((b, mi, half))
                        pv = po.rearrange("p (h c) -> p h c", h=6)
                        rs = rs_p.tile([128, 6], bf16, tag="rs")
                        with nc.allow_low_precision(reason="softmax recip"):
                            nc.vector.reciprocal(rs[:msz], pv[:msz, :, 64])
                            ov = o_tok[b][mi].rearrange(
                                "p (h c) -> p h c", h=12)
                            nc.vector.tensor_mul(
                                ov[:msz, 6 * half:6 * half + 6, :],
                                pv[:msz, :, 0:64],
                                rs[:msz, :, None].broadcast_to([msz, 6, 64]),
                            )
            # transpose a group of 2 o_tok column-tiles -> oT (feature-major)
            def t_group(b, g):
                c0 = b * N_TOK
                mi = g // 3
                m0, msz = TOK_TILES[mi]
                for j in range(2):
                    kt = (g % 3) * 2 + j
                    ot = ps_ot.tile([128, 128], bf16, tag="ot")
                    nc.tensor.transpose(
                        ot[:, :msz],
                        o_tok[b][mi][:msz, kt * 128:(kt + 1) * 128],
                        idn_sb[:msz, :msz],
                    )
                    if b >= 6:
                        nc.scalar.activation(
                            oT_sb[kt][:, c0 + m0:c0 + m0 + msz], ot[:, :msz],
                            AF.Copy)
                    else:
                        nc.vector.tensor_copy(
                            oT_sb[kt][:, c0 + m0:c0 + m0 + msz], ot[:, :msz])

            # ---- stage D: y = O @ Wp^T + bias; DMA out ----
            # token tiles packed across batch boundaries (13 x 128 instead of
            # 8 x (128+68)): DRAM rows are contiguous over (b, t), so each
            # tile DMAs out in 1-2 per-batch pieces
            def d_unit(tt, ti, nt):
                t0 = tt * 128
                tsz = min(128, NTB - t0)
                ps = ps_big.tile([128, NT2], f32, tag="big", name="ps")
                for kt in range(6):
                    nc.tensor.matmul(
                        ps[:tsz, :384],
                        oT_sb[kt][:, t0:t0 + tsz],
                        wp_sb[kt][:, nt * 384:(nt + 1) * 384],
                        start=(kt == 0), stop=(kt == 5),
                    )
                y_sb = y_p.tile([128, 384], f32, tag="y", name="y_sb")
                nc.vector.tensor_add(
                    y_sb[:tsz], ps[:tsz, :384],
                    bias_sb[:tsz, nt * 384:(nt + 1) * 384])
                r0 = t0
                while r0 < t0 + tsz:
                    b = r0 // N_TOK
                    r1 = min((b + 1) * N_TOK, t0 + tsz)
                    nc.sync.dma_start(
                        out=out_d[b, r0 - b * N_TOK:r1 - b * N_TOK,
                                  nt * 384:(nt + 1) * 384],
                        in_=y_sb[r0 - t0:r1 - t0])
                    r0 = r1

            # ---- schedule ----
            for kt in range(6):
                a_iter2(0, kt)
            for kt in range(6):
                a_iter2(1, kt)
            for kt in range(6):
                a_iter2(2, kt)
            for kt in range(6):
                a_iter2(3, kt)
            for nt in range(4):
                for mt in range(6):
                    k_group(mt, nt)

            for nt in range(4):
                for mt in range(6):
                    q_group(mt, nt)
            for b in (0, 1):
                for ti in range(2):
                    for nt in range(2):
                        v_unit(b, ti, nt)

            # C with v(2..7), deferred transposes, and D(b) units
            # interleaved between pairs (2 fillers per pair).
            from collections import deque
            fillers = deque()
            for b in range(2, B_PER_CORE):
                for ti in range(2):
                    for nt in range(2):
                        fillers.append(("v", b, ti, nt))

            def pop_fill(n):
                for _ in range(n):
                    if not fillers:
                        return
                    kind, fb, i1, i2 = fillers.popleft()
                    if kind == "v":
                        v_unit(fb, i1, i2)
                    elif kind == "t":
                        t_group(fb, i1)
                    else:
                        d_unit(fb, i1, i2)

            seq = [(b, p) for b in range(B_PER_CORE) for p in range(6)]
            c_front(*seq[0])
            for i, (b, p) in enumerate(seq):
                if i + 1 < len(seq):
                    c_front(*seq[i + 1])
                c_back(b, p)
                if p == 5:
                    for g in range(6):
                        fillers.append(("t", b, g, 0))
                    # D token-tiles whose last contributing batch is b
                    for tt in range(13):
                        tsz = min(128, NTB - tt * 128)
                        if (tt * 128 + tsz - 1) // N_TOK == b:
                            for nt in range(2):
                                fillers.append(("d", tt, 0, nt))
                pop_fill(2 if len(fillers) > 16 else 1)
            pop_fill(10**9)

    return nc


_CACHED_NC = None


def kernel(x, w_qkv, w_proj, b_proj, factors):
    global LAST_EXEC_NS, LAST_TRACE, _CACHED_NC
    from concourse.bass_utils import run_bass_kernel_spmd

    factors = np.asarray(factors, dtype=np.float32)
    scale = HEAD_DIM ** -0.5
    G_s = _grid_g(factors) * scale

    w_qkv = np.asarray(w_qkv, dtype=np.float32)
    in_common = {
        "gT": np.ascontiguousarray(G_s.T).astype(BF16),
        "wqT": np.ascontiguousarray(w_qkv[0:DIM, :].T).astype(BF16),
        "wkT": np.ascontiguousarray(w_qkv[DIM:2 * DIM, :].T).astype(BF16),
        "wvT": np.ascontiguousarray(w_qkv[2 * DIM:3 * DIM, :].T).astype(BF16),
        "wpT": np.ascontiguousarray(np.asarray(w_proj, dtype=np.float32).T).astype(BF16),
        "bias": np.asarray(b_proj, dtype=np.float32),
        "idn": np.eye(128, dtype=np.float32).astype(BF16),
    }
    x = np.asarray(x, dtype=np.float32).astype(BF16)
    in_maps = []
    for c in range(N_CORES):
        xc = x[c * B_PER_CORE:(c + 1) * B_PER_CORE]
        xTc = np.ascontiguousarray(
            xc.reshape(NTB, DIM).T)
        in_maps.append({"x": xc, "xT": xTc, **in_common})

    if _CACHED_NC is None:
        _CACHED_NC = _build_bass()
    nc = _CACHED_NC

    trace = bool(int(os.environ.get("KERNEL_TRACE", "0")))
    res = run_bass_kernel_spmd(nc, in_maps, core_ids=list(range(N_CORES)),
                               trace=trace)
    LAST_EXEC_NS = res.exec_time_ns
    if res.instructions_and_trace is not None:
        LAST_TRACE = res.instructions_and_trace[1]
    out = np.concatenate([res.results[c]["out"] for c in range(N_CORES)], axis=0)
    return out.astype(np.float32)
